# revision 16
# baseline (speedup 1.0000x reference)
"""Trainium2 Bass kernel for nn_CLF_block (channel-attention block).

Reference computation (per batch item i, with x = concat([a,b], ch) in [256, N],
N = H*W = 16384):
    z  = w1 x + b1 1^T
    q  = w2 z + b2 1^T ;  k = w3 z + b3 1^T ;  v = w4 z + b4 1^T
    qk = q k^T ; attn = softmax(qk, -1) ; out = attn v

Host-side weight folding (free: runs in numpy inside kernel()):
    q = A x + p 1^T   with A = w2 w1, p = w2 b1 + b2
    k = B x + r 1^T   with B = w3 w1, r = w3 b1 + b3
    v = D x + t 1^T   with D = w4 w1, t = w4 b1 + b4
so with Gx = x x^T (symmetric) and sx = x 1:
    qk   = A Gx B^T + (A sx) r^T + p (B sx)^T + N p r^T
    attn = softmax(qk)
    out  = (attn D) x + (attn t) 1^T = W x + c0 1^T

Numerics: x is rounded to fp16 on host; Gx accumulates fp16 products in f32
(PSUM), the 256x256 algebra runs in f32, W and the pass-2 matmul run in fp16,
and the output is stored as fp16 (upcast on host). Measured end-to-end error
vs the f64 reference: ~2.9e-3 max-rel (tolerance 2e-2).

Per-core HBM traffic: 8.4 MiB x^T stream (pass 1) + 8.4 MiB resident x
(pass 2) + 8.4 MiB output + ~0.8 MiB weights ~= 26 MiB -> memory-bound at
~73 us. DMA order is arranged so the pass-1 stream goes first, constants
early, the pass-2 resident load fills the DMA idle during the algebra phase,
and output stores ride a separate queue.

Sharding: data-parallel over batch, one batch item per NeuronCore (B=8).
"""

import sys

if "/opt/trn_rl_repo" not in sys.path:
    sys.path.insert(0, "/opt/trn_rl_repo")

from contextlib import ExitStack

import numpy as np

import concourse.bass as bass
import concourse.mybir as mybir
import concourse.tile as tile
from concourse import bacc
from concourse.bass_utils import run_bass_kernel_spmd

F32 = mybir.dt.float32
F16 = mybir.dt.float16
P = 128            # partitions / channel block
C = 256            # channels
NPIX = 128 * 128   # spatial positions per batch item
NPIECE = 16        # x^T stream pieces
CH_PP = 8          # gram chunks per piece
NCHUNK = NPIECE * CH_PP   # 128 gram chunks
XCHUNK = NPIX // 2        # resident x DMA chunk width
OUTW = 4096        # output staging tile width
NT = 512           # pass-2 psum tile width


def _emit(nc, tc, ctx, d_in, d_out):
    """Emit the Tile program for one core (one batch item)."""
    xht_d, xh_d = d_in["xht"], d_in["xh"]
    wcat, dmat, brows, bcols, ident = (d_in["wcat"], d_in["dmat"],
                                       d_in["brows"], d_in["bcols"],
                                       d_in["ident"])
    out_d = d_out["out"]

    const = ctx.enter_context(tc.tile_pool(name="const", bufs=1))
    xpool = ctx.enter_context(tc.tile_pool(name="xpool", bufs=1))

    # --- PE warm-up: ~3.4us of matmuls on a zeroed tile so the HAM clock
    # gate is already released when the first stream piece lands.
    warm16 = const.tile([P, C], F16, name="warm16", tag="warm16")
    nc.vector.memset(warm16, 0.0)
    with tc.tile_pool(name="warm_ps0", bufs=1, space="PSUM") as wps0:
        wp = wps0.tile([P, C], F32, name="wp0", tag="wp0")
        for _ in range(16):
            nc.tensor.matmul(wp, warm16[:, 0:P], warm16,
                             start=True, stop=True)

    # preload the EXP activation table so the softmax doesn't pay the
    # 1.3us ACT_TABLE_LOAD on the critical path
    warm_act = const.tile([P, 4], F32, name="warm_act", tag="warm_act")
    nc.scalar.activation(out=warm_act, in_=warm16[:, 0:4],
                         func=mybir.ActivationFunctionType.Exp, bias=0.0)

    # --- pass-1 stream + constants + resident x, all FIFO on sync queue ---
    # First four stream pieces, then the small constants, then the remaining
    # pieces; the resident x chunks are issued last inside the pass-1 loop.
    xtp = ctx.enter_context(tc.tile_pool(name="xt_sb", bufs=5))
    xht_p = []
    for i in range(4):
        xt = xtp.tile([P, CH_PP, C + 1], F16, name="xht_p", tag="xht_p")
        nc.sync.dma_start(out=xt, in_=xht_d[i])
        xht_p.append(xt)


    # --- pass 1: Gx = xh xh^T (fp16 products, f32 accumulation) ----------
    # shh[b] accumulates rows b*128:(b+1)*128 of [Gx | sx] over all chunks.
    gx_sb = [
        const.tile([P, C + 1], F32, name=f"gx_sb{b}", tag=f"gx_sb{b}")
        for b in range(2)
    ]
    with tc.tile_pool(name="gx_ps", bufs=1, space="PSUM") as gxp:
        shh = [
            gxp.tile([P, C + 1], F32, name=f"shh{b}", tag=f"shh{b}")
            for b in range(2)
        ]
        for i in range(NPIECE):
            if i >= 4:
                xt = xtp.tile([P, CH_PP, C + 1], F16, name="xht_p",
                              tag="xht_p")
                nc.sync.dma_start(out=xt, in_=xht_d[i])
                xht_p.append(xt)
            for g in range(CH_PP):
                ch = i * CH_PP + g
                for b in range(2):
                    nc.tensor.matmul(shh[b],
                                     xht_p[i][:, g, b * P:(b + 1) * P],
                                     xht_p[i][:, g, :],
                                     start=(ch == 0),
                                     stop=(ch == NCHUNK - 1))
        # constants: issued on the sync queue after the stream (they are
        # not needed until the algebra phase; issuing them earlier steals
        # stream bandwidth and starves pass-1)
        w_sb = []
        for k in range(2):
            wt = const.tile([P, 2 * C], F32, name=f"w_sb{k}", tag=f"w_sb{k}")
            nc.sync.dma_start(out=wt, in_=wcat[k * P:(k + 1) * P, :])
            w_sb.append(wt)
        at_ = [w_sb[k][:, 0 * C:1 * C] for k in range(2)]   # A^T  [c, o]
        bt_ = [w_sb[k][:, 1 * C:2 * C] for k in range(2)]   # B^T  [d, e]
        dm_ = []
        for k in range(2):
            dt_ = const.tile([P, C], F16, name=f"d_sb{k}", tag=f"d_sb{k}")
            nc.sync.dma_start(out=dt_, in_=dmat[k * P:(k + 1) * P, :])
            dm_.append(dt_)                                  # D [d, c] fp16
        rows = []
        for r in range(3):
            rt = const.tile([1, C], F32, name=f"brow{r}", tag=f"brow{r}")
            nc.sync.dma_start(out=rt, in_=brows[r:r + 1, :])
            rows.append(rt)
        p_row, r_row, nr_row = rows
        tcol = []
        for k in range(2):
            bt = const.tile([P, 1], F16, name=f"tcol{k}", tag=f"tcol{k}")
            nc.sync.dma_start(out=bt, in_=bcols[k * P:(k + 1) * P, :])
            tcol.append(bt)
        ident_sb = const.tile([P, P], F32, name="ident_sb", tag="ident_sb")
        nc.sync.dma_start(out=ident_sb, in_=ident[:, :])

        # resident x for pass 2, after the stream on the same queue
        xs = [[], []]
        for j in range(2):
            for k in range(2):
                xr = xpool.tile([P, XCHUNK], F16, name=f"x{k}_{j}",
                                tag=f"x{k}_{j}")
                nc.sync.dma_start(
                    out=xr,
                    in_=xh_d[k * P:(k + 1) * P,
                             j * XCHUNK:(j + 1) * XCHUNK])
                xs[k].append(xr)
        for b in range(2):
            nc.vector.tensor_copy(gx_sb[b], shh[b])

    # --- 256x256 algebra --------------------------------------------------
    alg = const
    with tc.tile_pool(name="alg_ps", bufs=3, space="PSUM") as ap:
        # asx_row = (A sx)^T, bsx_row = (B sx)^T
        asx_row = alg.tile([1, C], F32, name="asx_row", tag="asx_row")
        bsx_row = alg.tile([1, C], F32, name="bsx_row", tag="bsx_row")
        for dst, wt in ((asx_row, at_), (bsx_row, bt_)):
            vps = ap.tile([1, C], F32, name="vps", tag="algsmall", bufs=2)
            for k in range(2):
                nc.tensor.matmul(vps, gx_sb[k][:, C:C + 1], wt[k],
                                 start=(k == 0), stop=(k == 1))
            nc.vector.tensor_copy(dst, vps)

        # S = Gx B^T (Gx symmetric: lhsT = Gx row-blocks)
        s_sb = []
        for b in range(2):
            sps = ap.tile([P, C], F32, name="sps", tag="alg")
            for k in range(2):
                nc.tensor.matmul(sps, gx_sb[k][:, b * P:(b + 1) * P],
                                 bt_[k], start=(k == 0), stop=(k == 1))
            st = alg.tile([P, C], F32, name=f"s_sb{b}", tag=f"s_sb{b}")
            nc.vector.tensor_copy(st, sps)
            s_sb.append(st)

        # qk = A S + (A sx) r^T + p (B sx)^T + N p r^T ; softmax rows
        attn_sb = []
        for b in range(2):
            qkps = ap.tile([P, C], F32, name="qkps", tag="alg")
            for k in range(2):
                nc.tensor.matmul(qkps, at_[k][:, b * P:(b + 1) * P],
                                 s_sb[k], start=(k == 0), stop=False)
            nc.tensor.matmul(qkps, asx_row[:, b * P:(b + 1) * P], r_row,
                             start=False, stop=False)
            nc.tensor.matmul(qkps, p_row[:, b * P:(b + 1) * P], bsx_row,
                             start=False, stop=False)
            nc.tensor.matmul(qkps, p_row[:, b * P:(b + 1) * P], nr_row,
                             start=False, stop=True)

            negmax = alg.tile([P, 1], F32, name=f"negmax{b}", tag=f"nm{b}")
            nc.vector.tensor_reduce(
                out=negmax, in_=qkps, op=mybir.AluOpType.max,
                axis=mybir.AxisListType.X, negate=True,
            )
            expq = alg.tile([P, C], F32, name=f"expq{b}", tag=f"expq{b}")
            denom = alg.tile([P, 1], F32, name=f"denom{b}", tag=f"dn{b}")
            nc.scalar.activation(
                out=expq, in_=qkps, func=mybir.ActivationFunctionType.Exp,
                bias=negmax, scale=1.0, accum_out=denom,
            )
            rden = alg.tile([P, 1], F32, name=f"rden{b}", tag=f"rd{b}")
            nc.vector.reciprocal(rden, denom)
            at = alg.tile([P, C], F32, name=f"attn{b}", tag=f"attn{b}")
            nc.vector.tensor_scalar_mul(at, expq, rden)
            attn_sb.append(at)

        # keep-warm: PE would otherwise idle >3.4us waiting on the softmax
        # chain and get HAM-throttled for the start of pass 2.
        warm_ps = ap.tile([P, C], F32, name="warm_ps", tag="warm", bufs=1)
        for _ in range(6):
            nc.tensor.matmul(warm_ps, gx_sb[0][:, 0:P], bt_[0],
                             start=True, stop=True)

        # attn^T (4 PE transposes), stored fp16 for the cheap fp16 W/c0 mms
        attnT_sb = [
            alg.tile([P, C], F16, name=f"attnT{j}", tag=f"attnT{j}")
            for j in range(2)
        ]
        for b in range(2):
            for j in range(2):
                tps = ap.tile([P, P], F32, name="tps", tag="algtp", bufs=2)
                nc.tensor.transpose(tps, attn_sb[b][:, j * P:(j + 1) * P],
                                    ident_sb)
                nc.vector.tensor_copy(attnT_sb[j][:, b * P:(b + 1) * P], tps)

        # W^T = D^T attn^T, cast to fp16 for pass 2
        wt16 = []
        for b in range(2):
            wps = ap.tile([P, C], F32, name="wps", tag="alg")
            for k in range(2):
                nc.tensor.matmul(wps, dm_[k][:, b * P:(b + 1) * P],
                                 attnT_sb[k], start=(k == 0), stop=(k == 1))
            wt_ = alg.tile([P, C], F16, name=f"wt16_{b}", tag=f"wt16_{b}")
            nc.vector.tensor_copy(wt_, wps)
            wt16.append(wt_)

        # c0 = attn t (per q block)
        c0_col = []
        for b in range(2):
            cps = ap.tile([P, 1], F32, name="cps", tag="algsmall", bufs=2)
            for k in range(2):
                nc.tensor.matmul(cps, attnT_sb[k][:, b * P:(b + 1) * P],
                                 tcol[k], start=(k == 0), stop=(k == 1))
            ct = alg.tile([P, 1], F32, name=f"c0_col{b}", tag=f"c0_col{b}")
            nc.vector.tensor_copy(ct, cps)
            c0_col.append(ct)
        for _ in range(3):
            nc.tensor.matmul(warm_ps, gx_sb[0][:, 0:P], bt_[0],
                             start=True, stop=True)

    # --- pass 2: out = W x + c0 1^T, fp16, stores on scalar queue --------
    PW = 2 * NT   # psum tile spans 2 banks; one wide drain per tile
    with tc.tile_pool(name="o_ps", bufs=4, space="PSUM") as ops, \
         tc.tile_pool(name="o_sb", bufs=3) as osb:
        for i in range(NPIX // OUTW):
            xj = (i * OUTW) // XCHUNK
            xo = (i * OUTW) % XCHUNK
            for b in range(2):
                ot = osb.tile([P, OUTW], F16, name="ot", tag="ot")
                for t in range(OUTW // PW):
                    pst = ops.tile([P, PW], F32, name="pst", tag="pst")
                    for s in range(2):
                        for k in range(2):
                            nc.tensor.matmul(
                                pst[:, s * NT:(s + 1) * NT],
                                wt16[k][:, b * P:(b + 1) * P],
                                xs[k][xj][:, xo + t * PW + s * NT:
                                          xo + t * PW + (s + 1) * NT],
                                start=(k == 0),
                                stop=(k == 1),
                            )
                    # split psum drain (bias add + fp16 cast) across the
                    # otherwise-idle Scalar and Vector engines
                    if t % 2 == 0:
                        nc.scalar.activation(
                            out=ot[:, t * PW:(t + 1) * PW], in_=pst,
                            func=mybir.ActivationFunctionType.Identity,
                            bias=c0_col[b], scale=1.0,
                        )
                    else:
                        nc.vector.tensor_scalar_add(
                            ot[:, t * PW:(t + 1) * PW], pst, c0_col[b],
                        )
                nc.scalar.dma_start(
                    out=out_d[b * P:(b + 1) * P, i * OUTW:(i + 1) * OUTW],
                    in_=ot,
                )


def build_program(enable_asserts=False):
    nc = bacc.Bacc(
        "TRN2",
        target_bir_lowering=False,
        debug=False,
        enable_asserts=enable_asserts,
        num_devices=8,
    )
    d_in = {
        "xht": nc.dram_tensor("xht", [NPIECE, P, CH_PP, C + 1],
                              F16, kind="ExternalInput").ap(),
        "xh": nc.dram_tensor("xh", [C, NPIX], F16,
                             kind="ExternalInput").ap(),
        "wcat": nc.dram_tensor("wcat", [C, 2 * C], F32,
                               kind="ExternalInput").ap(),
        "dmat": nc.dram_tensor("dmat", [C, C], F16,
                               kind="ExternalInput").ap(),
        "brows": nc.dram_tensor("brows", [3, C], F32,
                                kind="ExternalInput").ap(),
        "bcols": nc.dram_tensor("bcols", [C, 1], F16,
                                kind="ExternalInput").ap(),
        "ident": nc.dram_tensor("ident", [P, P], F32,
                                kind="ExternalInput").ap(),
    }
    d_out = {
        "out": nc.dram_tensor("out", [C, NPIX], F16,
                              kind="ExternalOutput").ap(),
    }
    with tile.TileContext(nc) as tc, ExitStack() as ctx:
        _emit(nc, tc, ctx, d_in, d_out)
    nc.compile()
    return nc


def make_in_maps(a, b, w1, b1, w2, b2, w3, b3, w4, b4):
    N = NPIX
    f = np.float32
    f64 = np.float64
    A = (w2.astype(f64) @ w1.astype(f64))
    B_ = (w3.astype(f64) @ w1.astype(f64))
    D = (w4.astype(f64) @ w1.astype(f64))
    p = (w2.astype(f64) @ b1.astype(f64) + b2)
    r = (w3.astype(f64) @ b1.astype(f64) + b3)
    t = (w4.astype(f64) @ b1.astype(f64) + b4)
    wcat = np.concatenate([A.T, B_.T], axis=1).astype(f)
    dmat = D.astype(np.float16)
    brows = np.stack([p, r, N * r]).astype(f)
    bcols = t[:, None].astype(np.float16)
    ident = np.eye(P, dtype=f)
    B = a.shape[0]
    in_maps = []
    for i in range(B):
        x = np.concatenate([a[i].reshape(P, N), b[i].reshape(P, N)], axis=0)
        xh = x.astype(np.float16)
        xht = np.ascontiguousarray(
            xh.T.reshape(NPIECE, CH_PP, P, C).transpose(0, 2, 1, 3))
        ones = np.ones((NPIECE, P, CH_PP, 1), np.float16)
        xht = np.ascontiguousarray(np.concatenate([xht, ones], axis=3))
        in_maps.append({
            "xht": xht,
            "xh": xh,
            "wcat": wcat,
            "dmat": dmat,
            "brows": brows,
            "bcols": bcols,
            "ident": ident,
        })
    return in_maps


_CACHE = {}


def kernel(a, b, w1, b1, w2, b2, w3, b3, w4, b4, _trace=False):
    a = np.asarray(a, dtype=np.float32)
    b = np.asarray(b, dtype=np.float32)
    args = [np.asarray(t, dtype=np.float32)
            for t in (w1, b1, w2, b2, w3, b3, w4, b4)]
    if "nc" not in _CACHE:
        _CACHE["nc"] = build_program()
    nc = _CACHE["nc"]
    in_maps = make_in_maps(a, b, *args)
    res = run_bass_kernel_spmd(nc, in_maps, core_ids=list(range(8)),
                               trace=_trace)
    B, Ch, H, W = a.shape
    out = np.stack([
        r["out"].astype(np.float32).reshape(C, H, W) for r in res.results
    ])
    if _trace:
        _CACHE["last_results"] = res
    return out


# revision 18
# speedup vs baseline: 1.2461x; 1.2461x over previous
"""Trainium2 Bass kernel for nn_CLF_block (channel-attention block).

Reference computation (per batch item i, with x = concat([a,b], ch) in [256, N],
N = H*W = 16384):
    z  = w1 x + b1 1^T
    q  = w2 z + b2 1^T ;  k = w3 z + b3 1^T ;  v = w4 z + b4 1^T
    qk = q k^T ; attn = softmax(qk, -1) ; out = attn v

Host-side weight folding (free: runs in numpy inside kernel()):
    q = A x + p 1^T   with A = w2 w1, p = w2 b1 + b2
    k = B x + r 1^T   with B = w3 w1, r = w3 b1 + b3
    v = D x + t 1^T   with D = w4 w1, t = w4 b1 + b4
so with Gx = x x^T (symmetric) and sx = x 1:
    qk   = A Gx B^T + (A sx) r^T + p (B sx)^T + N p r^T
    attn = softmax(qk)
    out  = (attn D) x + (attn t) 1^T = W x + c0 1^T

Numerics: x is rounded to fp16 on host; Gx accumulates fp16 products in f32
(PSUM), the 256x256 algebra runs in f32, W and the pass-2 matmul run in fp16,
and the output is stored as fp16 (upcast on host). Measured end-to-end error
vs the f64 reference: ~2.9e-3 max-rel (tolerance 2e-2).

Per-core HBM traffic: 8.4 MiB x^T stream (pass 1) + 8.4 MiB resident x
(pass 2) + 8.4 MiB output + ~0.8 MiB weights ~= 26 MiB -> memory-bound at
~73 us. DMA order is arranged so the pass-1 stream goes first, constants
early, the pass-2 resident load fills the DMA idle during the algebra phase,
and output stores ride a separate queue.

Sharding: data-parallel over batch, one batch item per NeuronCore (B=8).
"""

import sys

if "/opt/trn_rl_repo" not in sys.path:
    sys.path.insert(0, "/opt/trn_rl_repo")

from contextlib import ExitStack

import numpy as np

import concourse.bass as bass
import concourse.mybir as mybir
import concourse.tile as tile
from concourse import bacc
from concourse.bass_utils import run_bass_kernel_spmd

F32 = mybir.dt.float32
F16 = mybir.dt.float16
P = 128            # partitions / channel block
C = 256            # channels
NPIX = 128 * 128   # spatial positions per batch item
NPIECE = 16        # x^T stream pieces
CH_PP = 8          # gram chunks per piece
NCHUNK = NPIECE * CH_PP   # 128 gram chunks
XCHUNK = NPIX // 2        # resident x DMA chunk width
OUTW = 4096        # output staging tile width
NT = 512           # pass-2 psum tile width


def _emit(nc, tc, ctx, d_in, d_out):
    """Emit the Tile program for one core (one batch item)."""
    xht_d, xh_d = d_in["xht"], d_in["xh"]
    wcat, dmat, brows, bcols, ident = (d_in["wcat"], d_in["dmat"],
                                       d_in["brows"], d_in["bcols"],
                                       d_in["ident"])
    out_d = d_out["out"]

    const = ctx.enter_context(tc.tile_pool(name="const", bufs=1))
    xpool = ctx.enter_context(tc.tile_pool(name="xpool", bufs=1))

    # --- PE warm-up: ~3.4us of matmuls on a zeroed tile so the HAM clock
    # gate is already released when the first stream piece lands.
    warm16 = const.tile([P, C], F16, name="warm16", tag="warm16")
    nc.vector.memset(warm16, 0.0)
    with tc.tile_pool(name="warm_ps0", bufs=1, space="PSUM") as wps0:
        wp = wps0.tile([P, C], F32, name="wp0", tag="wp0")
        for _ in range(16):
            nc.tensor.matmul(wp, warm16[:, 0:P], warm16,
                             start=True, stop=True)

    # preload the EXP activation table so the softmax doesn't pay the
    # 1.3us ACT_TABLE_LOAD on the critical path
    warm_act = const.tile([P, 4], F32, name="warm_act", tag="warm_act")
    nc.scalar.activation(out=warm_act, in_=warm16[:, 0:4],
                         func=mybir.ActivationFunctionType.Exp, bias=0.0)

    # --- pass-1 stream + constants + resident x, all FIFO on sync queue ---
    # First four stream pieces, then the small constants, then the remaining
    # pieces; the resident x chunks are issued last inside the pass-1 loop.
    xtp = ctx.enter_context(tc.tile_pool(name="xt_sb", bufs=5))
    H_PP = CH_PP // 2
    xh0 = []
    for h in range(2):
        xt = const.tile([P, H_PP, C + 1], F16, name=f"xh0_{h}",
                        tag=f"xh0_{h}")
        nc.sync.dma_start(out=xt, in_=xht_d[0][:, h * H_PP:(h + 1) * H_PP, :])
        xh0.append(xt)
    xht_p = [None]
    for i in range(1, 4):
        xt = xtp.tile([P, CH_PP, C + 1], F16, name="xht_p", tag="xht_p")
        nc.sync.dma_start(out=xt, in_=xht_d[i])
        xht_p.append(xt)


    # --- pass 1: Gx = xh xh^T (fp16 products, f32 accumulation) ----------
    # shh[b] accumulates rows b*128:(b+1)*128 of [Gx | sx] over all chunks.
    gx_sb = [
        const.tile([P, C + 1], F32, name=f"gx_sb{b}", tag=f"gx_sb{b}")
        for b in range(2)
    ]
    cst = {}

    def emit_consts():
        # constants on the sync queue mid-stream: issued while the piece
        # pipeline is throttled by PE consumption, landing well before the
        # algebra phase needs them without starving the stream head.
        w_sb = []
        for k in range(2):
            wt = const.tile([P, 2 * C], F32, name=f"w_sb{k}", tag=f"w_sb{k}")
            nc.sync.dma_start(out=wt, in_=wcat[k * P:(k + 1) * P, :])
            w_sb.append(wt)
        cst["at_"] = [w_sb[k][:, 0 * C:1 * C] for k in range(2)]  # A^T [c,o]
        cst["bt_"] = [w_sb[k][:, 1 * C:2 * C] for k in range(2)]  # B^T [d,e]
        dm_ = []
        for k in range(2):
            dt_ = const.tile([P, C], F16, name=f"d_sb{k}", tag=f"d_sb{k}")
            nc.sync.dma_start(out=dt_, in_=dmat[k * P:(k + 1) * P, :])
            dm_.append(dt_)
        cst["dm_"] = dm_                                     # D [d, c] fp16
        rows = []
        for r in range(3):
            rt = const.tile([1, C], F32, name=f"brow{r}", tag=f"brow{r}")
            nc.sync.dma_start(out=rt, in_=brows[r:r + 1, :])
            rows.append(rt)
        cst["rows"] = rows
        tcol = []
        for k in range(2):
            bt = const.tile([P, 1], F16, name=f"tcol{k}", tag=f"tcol{k}")
            nc.sync.dma_start(out=bt, in_=bcols[k * P:(k + 1) * P, :])
            tcol.append(bt)
        cst["tcol"] = tcol
        ident_sb = const.tile([P, P], F32, name="ident_sb", tag="ident_sb")
        nc.sync.dma_start(out=ident_sb, in_=ident[:, :])
        cst["ident_sb"] = ident_sb

    with tc.tile_pool(name="gx_ps", bufs=1, space="PSUM") as gxp:
        shh = [
            gxp.tile([P, C + 1], F32, name=f"shh{b}", tag=f"shh{b}")
            for b in range(2)
        ]
        for i in range(NPIECE):
            if i >= 4:
                xt = xtp.tile([P, CH_PP, C + 1], F16, name="xht_p",
                              tag="xht_p")
                nc.sync.dma_start(out=xt, in_=xht_d[i])
                xht_p.append(xt)
            for g in range(CH_PP):
                ch = i * CH_PP + g
                src_t = (xh0[g // H_PP][:, g % H_PP] if i == 0
                         else xht_p[i][:, g])
                for b in range(2):
                    nc.tensor.matmul(shh[b],
                                     src_t[:, b * P:(b + 1) * P],
                                     src_t,
                                     start=(ch == 0),
                                     stop=(ch == NCHUNK - 1))
            if i == 10:
                emit_consts()
        # resident x for pass 2, after the stream on the same queue
        xs = [[], []]
        for j in range(2):
            for k in range(2):
                xr = xpool.tile([P, XCHUNK], F16, name=f"x{k}_{j}",
                                tag=f"x{k}_{j}")
                nc.sync.dma_start(
                    out=xr,
                    in_=xh_d[k * P:(k + 1) * P,
                             j * XCHUNK:(j + 1) * XCHUNK])
                xs[k].append(xr)
        for b in range(2):
            nc.vector.tensor_copy(gx_sb[b], shh[b])

    # --- 256x256 algebra --------------------------------------------------
    alg = const
    at_, bt_, dm_ = cst["at_"], cst["bt_"], cst["dm_"]
    p_row, r_row, nr_row = cst["rows"]
    tcol, ident_sb = cst["tcol"], cst["ident_sb"]
    with tc.tile_pool(name="alg_ps", bufs=3, space="PSUM") as ap:
        # asx_row = (A sx)^T, bsx_row = (B sx)^T
        asx_row = alg.tile([1, C], F32, name="asx_row", tag="asx_row")
        bsx_row = alg.tile([1, C], F32, name="bsx_row", tag="bsx_row")
        for dst, wt in ((asx_row, at_), (bsx_row, bt_)):
            vps = ap.tile([1, C], F32, name="vps", tag="algsmall", bufs=2)
            for k in range(2):
                nc.tensor.matmul(vps, gx_sb[k][:, C:C + 1], wt[k],
                                 start=(k == 0), stop=(k == 1))
            nc.vector.tensor_copy(dst, vps)

        # S = Gx B^T (Gx symmetric: lhsT = Gx row-blocks)
        s_sb = []
        for b in range(2):
            sps = ap.tile([P, C], F32, name="sps", tag="alg")
            for k in range(2):
                nc.tensor.matmul(sps, gx_sb[k][:, b * P:(b + 1) * P],
                                 bt_[k], start=(k == 0), stop=(k == 1))
            st = alg.tile([P, C], F32, name=f"s_sb{b}", tag=f"s_sb{b}")
            nc.vector.tensor_copy(st, sps)
            s_sb.append(st)

        # qk = A S + (A sx) r^T + p (B sx)^T + N p r^T ; softmax rows
        attn_sb = []
        for b in range(2):
            qkps = ap.tile([P, C], F32, name="qkps", tag="alg")
            for k in range(2):
                nc.tensor.matmul(qkps, at_[k][:, b * P:(b + 1) * P],
                                 s_sb[k], start=(k == 0), stop=False)
            nc.tensor.matmul(qkps, asx_row[:, b * P:(b + 1) * P], r_row,
                             start=False, stop=False)
            nc.tensor.matmul(qkps, p_row[:, b * P:(b + 1) * P], bsx_row,
                             start=False, stop=False)
            nc.tensor.matmul(qkps, p_row[:, b * P:(b + 1) * P], nr_row,
                             start=False, stop=True)

            negmax = alg.tile([P, 1], F32, name=f"negmax{b}", tag=f"nm{b}")
            nc.vector.tensor_reduce(
                out=negmax, in_=qkps, op=mybir.AluOpType.max,
                axis=mybir.AxisListType.X, negate=True,
            )
            expq = alg.tile([P, C], F32, name=f"expq{b}", tag=f"expq{b}")
            denom = alg.tile([P, 1], F32, name=f"denom{b}", tag=f"dn{b}")
            nc.scalar.activation(
                out=expq, in_=qkps, func=mybir.ActivationFunctionType.Exp,
                bias=negmax, scale=1.0, accum_out=denom,
            )
            rden = alg.tile([P, 1], F32, name=f"rden{b}", tag=f"rd{b}")
            nc.vector.reciprocal(rden, denom)
            at = alg.tile([P, C], F32, name=f"attn{b}", tag=f"attn{b}")
            nc.vector.tensor_scalar_mul(at, expq, rden)
            attn_sb.append(at)

        # keep-warm: PE would otherwise idle >3.4us waiting on the softmax
        # chain and get HAM-throttled for the start of pass 2.
        warm_ps = ap.tile([P, C], F32, name="warm_ps", tag="warm", bufs=1)
        for _ in range(6):
            nc.tensor.matmul(warm_ps, gx_sb[0][:, 0:P], bt_[0],
                             start=True, stop=True)

        # attn^T (4 PE transposes), stored fp16 for the cheap fp16 W/c0 mms
        attnT_sb = [
            alg.tile([P, C], F16, name=f"attnT{j}", tag=f"attnT{j}")
            for j in range(2)
        ]
        for b in range(2):
            for j in range(2):
                tps = ap.tile([P, P], F32, name="tps", tag="algtp", bufs=2)
                nc.tensor.transpose(tps, attn_sb[b][:, j * P:(j + 1) * P],
                                    ident_sb)
                nc.vector.tensor_copy(attnT_sb[j][:, b * P:(b + 1) * P], tps)

        # W^T = D^T attn^T, cast to fp16 for pass 2
        wt16 = []
        for b in range(2):
            wps = ap.tile([P, C], F32, name="wps", tag="alg")
            for k in range(2):
                nc.tensor.matmul(wps, dm_[k][:, b * P:(b + 1) * P],
                                 attnT_sb[k], start=(k == 0), stop=(k == 1))
            wt_ = alg.tile([P, C], F16, name=f"wt16_{b}", tag=f"wt16_{b}")
            nc.vector.tensor_copy(wt_, wps)
            wt16.append(wt_)

        # c0 = attn t (per q block)
        c0_col = []
        for b in range(2):
            cps = ap.tile([P, 1], F32, name="cps", tag="algsmall", bufs=2)
            for k in range(2):
                nc.tensor.matmul(cps, attnT_sb[k][:, b * P:(b + 1) * P],
                                 tcol[k], start=(k == 0), stop=(k == 1))
            ct = alg.tile([P, 1], F32, name=f"c0_col{b}", tag=f"c0_col{b}")
            nc.vector.tensor_copy(ct, cps)
            c0_col.append(ct)

    # --- pass 2: out = W x + c0 1^T, fp16, stores on scalar queue --------
    with tc.tile_pool(name="o_ps", bufs=8, space="PSUM") as ops, \
         tc.tile_pool(name="o_sb", bufs=3) as osb:
        nsub = OUTW // NT
        for i in range(NPIX // OUTW):
            xj = (i * OUTW) // XCHUNK
            xo = (i * OUTW) % XCHUNK
            for b in range(2):
                ot = osb.tile([P, OUTW], F16, name="ot", tag="ot")
                for t in range(nsub):
                    pst = ops.tile([P, NT], F32, name="pst", tag="pst")
                    for k in range(2):
                        nc.tensor.matmul(
                            pst,
                            wt16[k][:, b * P:(b + 1) * P],
                            xs[k][xj][:, xo + t * NT:xo + (t + 1) * NT],
                            start=(k == 0),
                            stop=(k == 1),
                        )
                    # psum drain (bias add + fp16 cast) split across the
                    # otherwise-idle Scalar and Vector engines (GpSimd
                    # cannot read PSUM)
                    if t % 2 == 0:
                        nc.scalar.activation(
                            out=ot[:, t * NT:(t + 1) * NT], in_=pst,
                            func=mybir.ActivationFunctionType.Identity,
                            bias=c0_col[b], scale=1.0,
                        )
                    else:
                        nc.vector.tensor_scalar_add(
                            ot[:, t * NT:(t + 1) * NT], pst, c0_col[b],
                        )
                # store via SWDGE on the idle GpSimd engine so the Scalar
                # engine's cycle budget stays dedicated to psum drains
                nc.gpsimd.dma_start(
                    out=out_d[b * P:(b + 1) * P, i * OUTW:(i + 1) * OUTW],
                    in_=ot,
                )


def build_program(enable_asserts=False):
    nc = bacc.Bacc(
        "TRN2",
        target_bir_lowering=False,
        debug=False,
        enable_asserts=enable_asserts,
        num_devices=8,
    )
    d_in = {
        "xht": nc.dram_tensor("xht", [NPIECE, P, CH_PP, C + 1],
                              F16, kind="ExternalInput").ap(),
        "xh": nc.dram_tensor("xh", [C, NPIX], F16,
                             kind="ExternalInput").ap(),
        "wcat": nc.dram_tensor("wcat", [C, 2 * C], F32,
                               kind="ExternalInput").ap(),
        "dmat": nc.dram_tensor("dmat", [C, C], F16,
                               kind="ExternalInput").ap(),
        "brows": nc.dram_tensor("brows", [3, C], F32,
                                kind="ExternalInput").ap(),
        "bcols": nc.dram_tensor("bcols", [C, 1], F16,
                                kind="ExternalInput").ap(),
        "ident": nc.dram_tensor("ident", [P, P], F32,
                                kind="ExternalInput").ap(),
    }
    d_out = {
        "out": nc.dram_tensor("out", [C, NPIX], F16,
                              kind="ExternalOutput").ap(),
    }
    with tile.TileContext(nc) as tc, ExitStack() as ctx:
        _emit(nc, tc, ctx, d_in, d_out)
    nc.compile()
    return nc


def make_in_maps(a, b, w1, b1, w2, b2, w3, b3, w4, b4):
    N = NPIX
    f = np.float32
    f64 = np.float64
    A = (w2.astype(f64) @ w1.astype(f64))
    B_ = (w3.astype(f64) @ w1.astype(f64))
    D = (w4.astype(f64) @ w1.astype(f64))
    p = (w2.astype(f64) @ b1.astype(f64) + b2)
    r = (w3.astype(f64) @ b1.astype(f64) + b3)
    t = (w4.astype(f64) @ b1.astype(f64) + b4)
    wcat = np.concatenate([A.T, B_.T], axis=1).astype(f)
    dmat = D.astype(np.float16)
    brows = np.stack([p, r, N * r]).astype(f)
    bcols = t[:, None].astype(np.float16)
    ident = np.eye(P, dtype=f)
    B = a.shape[0]
    in_maps = []
    for i in range(B):
        x = np.concatenate([a[i].reshape(P, N), b[i].reshape(P, N)], axis=0)
        xh = x.astype(np.float16)
        xht = np.ascontiguousarray(
            xh.T.reshape(NPIECE, CH_PP, P, C).transpose(0, 2, 1, 3))
        ones = np.ones((NPIECE, P, CH_PP, 1), np.float16)
        xht = np.ascontiguousarray(np.concatenate([xht, ones], axis=3))
        in_maps.append({
            "xht": xht,
            "xh": xh,
            "wcat": wcat,
            "dmat": dmat,
            "brows": brows,
            "bcols": bcols,
            "ident": ident,
        })
    return in_maps


_CACHE = {}


def kernel(a, b, w1, b1, w2, b2, w3, b3, w4, b4, _trace=False):
    a = np.asarray(a, dtype=np.float32)
    b = np.asarray(b, dtype=np.float32)
    args = [np.asarray(t, dtype=np.float32)
            for t in (w1, b1, w2, b2, w3, b3, w4, b4)]
    if "nc" not in _CACHE:
        _CACHE["nc"] = build_program()
    nc = _CACHE["nc"]
    in_maps = make_in_maps(a, b, *args)
    res = run_bass_kernel_spmd(nc, in_maps, core_ids=list(range(8)),
                               trace=_trace)
    B, Ch, H, W = a.shape
    out = np.stack([
        r["out"].astype(np.float32).reshape(C, H, W) for r in res.results
    ])
    if _trace:
        _CACHE["last_results"] = res
    return out


# revision 20
# speedup vs baseline: 1.2769x; 1.0247x over previous
"""Trainium2 Bass kernel for nn_CLF_block (channel-attention block).

Reference computation (per batch item i, with x = concat([a,b], ch) in [256, N],
N = H*W = 16384):
    z  = w1 x + b1 1^T
    q  = w2 z + b2 1^T ;  k = w3 z + b3 1^T ;  v = w4 z + b4 1^T
    qk = q k^T ; attn = softmax(qk, -1) ; out = attn v

Host-side weight folding (free: runs in numpy inside kernel()):
    q = A x + p 1^T   with A = w2 w1, p = w2 b1 + b2
    k = B x + r 1^T   with B = w3 w1, r = w3 b1 + b3
    v = D x + t 1^T   with D = w4 w1, t = w4 b1 + b4
so with Gx = x x^T (symmetric) and sx = x 1:
    qk   = A Gx B^T + (A sx) r^T + p (B sx)^T + N p r^T
    attn = softmax(qk)
    out  = (attn D) x + (attn t) 1^T = W x + c0 1^T

Numerics: x is rounded to fp16 on host; Gx accumulates fp16 products in f32
(PSUM), the 256x256 algebra runs in f32, W and the pass-2 matmul run in fp16,
and the output is stored as fp16 (upcast on host). Measured end-to-end error
vs the f64 reference: ~2.9e-3 max-rel (tolerance 2e-2).

Per-core HBM traffic: 8.4 MiB x^T stream (pass 1) + 8.4 MiB resident x
(pass 2) + 8.4 MiB output + ~0.8 MiB weights ~= 26 MiB -> memory-bound at
~73 us. DMA order is arranged so the pass-1 stream goes first, constants
early, the pass-2 resident load fills the DMA idle during the algebra phase,
and output stores ride a separate queue.

Sharding: data-parallel over batch, one batch item per NeuronCore (B=8).
"""

import sys

if "/opt/trn_rl_repo" not in sys.path:
    sys.path.insert(0, "/opt/trn_rl_repo")

from contextlib import ExitStack

import numpy as np

import concourse.bass as bass
import concourse.mybir as mybir
import concourse.tile as tile
from concourse import bacc
from concourse.bass_utils import run_bass_kernel_spmd

F32 = mybir.dt.float32
F16 = mybir.dt.float16
P = 128            # partitions / channel block
C = 256            # channels
NPIX = 128 * 128   # spatial positions per batch item
NPIECE = 16        # x^T stream pieces
CH_PP = 8          # gram chunks per piece
NCHUNK = NPIECE * CH_PP   # 128 gram chunks
XCHUNK = NPIX // 2        # resident x DMA chunk width
OUTW = 4096        # output staging tile width
NT = 512           # pass-2 psum tile width


def _emit(nc, tc, ctx, d_in, d_out):
    """Emit the Tile program for one core (one batch item)."""
    xht_d, xh_d = d_in["xht"], d_in["xh"]
    wcat, dmat, brows, bcols, ident = (d_in["wcat"], d_in["dmat"],
                                       d_in["brows"], d_in["bcols"],
                                       d_in["ident"])
    out_d = d_out["out"]

    const = ctx.enter_context(tc.tile_pool(name="const", bufs=1))
    xpool = ctx.enter_context(tc.tile_pool(name="xpool", bufs=1))

    # --- PE warm-up: ~3.4us of matmuls on a zeroed tile so the HAM clock
    # gate is already released when the first stream piece lands.
    warm16 = const.tile([P, C], F16, name="warm16", tag="warm16")
    nc.vector.memset(warm16, 0.0)
    with tc.tile_pool(name="warm_ps0", bufs=1, space="PSUM") as wps0:
        wp = wps0.tile([P, C], F32, name="wp0", tag="wp0")
        for _ in range(16):
            nc.tensor.matmul(wp, warm16[:, 0:P], warm16,
                             start=True, stop=True)

    # constants ride the scalar queue during the startup DMA-idle window
    # (before the first stream piece even lands), so they never contend
    # with the pass-1 stream
    cst = {}

    def emit_consts():
        w_sb = []
        for k in range(2):
            wt = const.tile([P, 2 * C], F32, name=f"w_sb{k}", tag=f"w_sb{k}")
            nc.scalar.dma_start(out=wt, in_=wcat[k * P:(k + 1) * P, :])
            w_sb.append(wt)
        cst["at_"] = [w_sb[k][:, 0 * C:1 * C] for k in range(2)]  # A^T [c,o]
        cst["bt_"] = [w_sb[k][:, 1 * C:2 * C] for k in range(2)]  # B^T [d,e]
        dm_ = []
        for k in range(2):
            dt_ = const.tile([P, C], F16, name=f"d_sb{k}", tag=f"d_sb{k}")
            nc.scalar.dma_start(out=dt_, in_=dmat[k * P:(k + 1) * P, :])
            dm_.append(dt_)
        cst["dm_"] = dm_                                     # D [d, c] fp16
        rows = []
        for r in range(3):
            rt = const.tile([1, C], F32, name=f"brow{r}", tag=f"brow{r}")
            nc.scalar.dma_start(out=rt, in_=brows[r:r + 1, :])
            rows.append(rt)
        cst["rows"] = rows
        tcol = []
        for k in range(2):
            bt = const.tile([P, 1], F16, name=f"tcol{k}", tag=f"tcol{k}")
            nc.scalar.dma_start(out=bt, in_=bcols[k * P:(k + 1) * P, :])
            tcol.append(bt)
        cst["tcol"] = tcol
        ident_sb = const.tile([P, P], F32, name="ident_sb", tag="ident_sb")
        nc.scalar.dma_start(out=ident_sb, in_=ident[:, :])
        cst["ident_sb"] = ident_sb

    emit_consts()

    # preload the EXP activation table so the softmax doesn't pay the
    # 1.3us ACT_TABLE_LOAD on the critical path
    warm_act = const.tile([P, 4], F32, name="warm_act", tag="warm_act")
    nc.scalar.activation(out=warm_act, in_=warm16[:, 0:4],
                         func=mybir.ActivationFunctionType.Exp, bias=0.0)

    # --- pass-1 stream + constants + resident x, all FIFO on sync queue ---
    # First four stream pieces, then the small constants, then the remaining
    # pieces; the resident x chunks are issued last inside the pass-1 loop.
    xtp = ctx.enter_context(tc.tile_pool(name="xt_sb", bufs=5))
    H_PP = CH_PP // 2
    xh0 = []
    for h in range(2):
        xt = const.tile([P, H_PP, C + 1], F16, name=f"xh0_{h}",
                        tag=f"xh0_{h}")
        nc.sync.dma_start(out=xt, in_=xht_d[0][:, h * H_PP:(h + 1) * H_PP, :])
        xh0.append(xt)
    xht_p = [None]
    for i in range(1, 4):
        xt = xtp.tile([P, CH_PP, C + 1], F16, name="xht_p", tag="xht_p")
        nc.sync.dma_start(out=xt, in_=xht_d[i])
        xht_p.append(xt)


    # --- pass 1: Gx = xh xh^T (fp16 products, f32 accumulation) ----------
    # shh[b] accumulates rows b*128:(b+1)*128 of [Gx | sx] over all chunks.
    gx_sb = [
        const.tile([P, C + 1], F32, name=f"gx_sb{b}", tag=f"gx_sb{b}")
        for b in range(2)
    ]
    with tc.tile_pool(name="gx_ps", bufs=1, space="PSUM") as gxp:
        shh = [
            gxp.tile([P, C + 1], F32, name=f"shh{b}", tag=f"shh{b}")
            for b in range(2)
        ]
        for i in range(NPIECE):
            if i >= 4:
                xt = xtp.tile([P, CH_PP, C + 1], F16, name="xht_p",
                              tag="xht_p")
                nc.sync.dma_start(out=xt, in_=xht_d[i])
                xht_p.append(xt)
            for g in range(CH_PP):
                ch = i * CH_PP + g
                src_t = (xh0[g // H_PP][:, g % H_PP] if i == 0
                         else xht_p[i][:, g])
                for b in range(2):
                    nc.tensor.matmul(shh[b],
                                     src_t[:, b * P:(b + 1) * P],
                                     src_t,
                                     start=(ch == 0),
                                     stop=(ch == NCHUNK - 1))
        # resident x for pass 2, after the stream on the same queue
        xs = [[], []]
        for j in range(2):
            for k in range(2):
                xr = xpool.tile([P, XCHUNK], F16, name=f"x{k}_{j}",
                                tag=f"x{k}_{j}")
                nc.sync.dma_start(
                    out=xr,
                    in_=xh_d[k * P:(k + 1) * P,
                             j * XCHUNK:(j + 1) * XCHUNK])
                xs[k].append(xr)
        # small sx columns first so asx/bsx matmuls can start immediately;
        # the big Gx copies run on Scalar and Vector in parallel
        sxc = []
        for b in range(2):
            sc = const.tile([P, 1], F32, name=f"sxc{b}", tag=f"sxc{b}")
            nc.vector.tensor_copy(sc, shh[b][:, C:C + 1])
            sxc.append(sc)
        nc.scalar.activation(out=gx_sb[0], in_=shh[0],
                             func=mybir.ActivationFunctionType.Identity,
                             bias=0.0, scale=1.0)
        nc.vector.tensor_copy(gx_sb[1], shh[1])

    # --- 256x256 algebra --------------------------------------------------
    alg = const
    at_, bt_, dm_ = cst["at_"], cst["bt_"], cst["dm_"]
    p_row, r_row, nr_row = cst["rows"]
    tcol, ident_sb = cst["tcol"], cst["ident_sb"]
    with tc.tile_pool(name="alg_ps", bufs=3, space="PSUM") as ap:
        # asx_row = (A sx)^T, bsx_row = (B sx)^T
        asx_row = alg.tile([1, C], F32, name="asx_row", tag="asx_row")
        bsx_row = alg.tile([1, C], F32, name="bsx_row", tag="bsx_row")
        for dst, wt in ((asx_row, at_), (bsx_row, bt_)):
            vps = ap.tile([1, C], F32, name="vps", tag="algsmall", bufs=2)
            for k in range(2):
                nc.tensor.matmul(vps, sxc[k], wt[k],
                                 start=(k == 0), stop=(k == 1))
            nc.vector.tensor_copy(dst, vps)
        bnr_row = alg.tile([1, C], F32, name="bnr_row", tag="bnr_row")
        nc.vector.tensor_add(bnr_row, bsx_row, nr_row)

        # S = Gx B^T (Gx symmetric: lhsT = Gx row-blocks)
        s_sb = []
        for b in range(2):
            sps = ap.tile([P, C], F32, name="sps", tag="alg")
            for k in range(2):
                nc.tensor.matmul(sps, gx_sb[k][:, b * P:(b + 1) * P],
                                 bt_[k], start=(k == 0), stop=(k == 1))
            st = alg.tile([P, C], F32, name=f"s_sb{b}", tag=f"s_sb{b}")
            nc.vector.tensor_copy(st, sps)
            s_sb.append(st)

        # qk = A S + (A sx) r^T + p (B sx)^T + N p r^T ; softmax rows
        attn_sb = []
        for b in range(2):
            qkps = ap.tile([P, C], F32, name="qkps", tag="alg")
            for k in range(2):
                nc.tensor.matmul(qkps, at_[k][:, b * P:(b + 1) * P],
                                 s_sb[k], start=(k == 0), stop=False)
            nc.tensor.matmul(qkps, asx_row[:, b * P:(b + 1) * P], r_row,
                             start=False, stop=False)
            nc.tensor.matmul(qkps, p_row[:, b * P:(b + 1) * P], bnr_row,
                             start=False, stop=True)

            negmax = alg.tile([P, 1], F32, name=f"negmax{b}", tag=f"nm{b}")
            nc.vector.tensor_reduce(
                out=negmax, in_=qkps, op=mybir.AluOpType.max,
                axis=mybir.AxisListType.X, negate=True,
            )
            expq = alg.tile([P, C], F32, name=f"expq{b}", tag=f"expq{b}")
            denom = alg.tile([P, 1], F32, name=f"denom{b}", tag=f"dn{b}")
            nc.scalar.activation(
                out=expq, in_=qkps, func=mybir.ActivationFunctionType.Exp,
                bias=negmax, scale=1.0, accum_out=denom,
            )
            rden = alg.tile([P, 1], F32, name=f"rden{b}", tag=f"rd{b}")
            nc.vector.reciprocal(rden, denom)
            # diag(1/denom): the transposes below fold the softmax
            # normalization into their stationary operand for free
            dident = alg.tile([P, P], F32, name=f"dident{b}", tag=f"di{b}")
            nc.vector.tensor_scalar_mul(dident, ident_sb, rden)
            attn_sb.append((expq, dident))

        # keep-warm: PE would otherwise idle >3.4us waiting on the softmax
        # chain and get HAM-throttled for the start of pass 2.
        warm_ps = ap.tile([P, C], F32, name="warm_ps", tag="warm", bufs=1)
        for _ in range(6):
            nc.tensor.matmul(warm_ps, gx_sb[0][:, 0:P], bt_[0],
                             start=True, stop=True)

        # attn^T (4 PE transposes), stored fp16 for the cheap fp16 W/c0 mms
        attnT_sb = [
            alg.tile([P, C], F16, name=f"attnT{j}", tag=f"attnT{j}")
            for j in range(2)
        ]
        for b in range(2):
            expq_b, dident_b = attn_sb[b]
            for j in range(2):
                tps = ap.tile([P, P], F32, name="tps", tag="algtp", bufs=2)
                nc.tensor.matmul(tps, expq_b[:, j * P:(j + 1) * P],
                                 dident_b, start=True, stop=True)
                nc.vector.tensor_copy(attnT_sb[j][:, b * P:(b + 1) * P], tps)

        # W^T = D^T attn^T, cast to fp16 for pass 2
        wt16 = []
        for b in range(2):
            wps = ap.tile([P, C], F32, name="wps", tag="alg")
            for k in range(2):
                nc.tensor.matmul(wps, dm_[k][:, b * P:(b + 1) * P],
                                 attnT_sb[k], start=(k == 0), stop=(k == 1))
            wt_ = alg.tile([P, C], F16, name=f"wt16_{b}", tag=f"wt16_{b}")
            if b == 0:
                nc.scalar.activation(
                    out=wt_, in_=wps,
                    func=mybir.ActivationFunctionType.Identity,
                    bias=0.0, scale=1.0)
            else:
                nc.vector.tensor_copy(wt_, wps)
            wt16.append(wt_)

        # c0 = attn t (per q block)
        c0_col = []
        for b in range(2):
            cps = ap.tile([P, 1], F32, name="cps", tag="algsmall", bufs=2)
            for k in range(2):
                nc.tensor.matmul(cps, attnT_sb[k][:, b * P:(b + 1) * P],
                                 tcol[k], start=(k == 0), stop=(k == 1))
            ct = alg.tile([P, 1], F32, name=f"c0_col{b}", tag=f"c0_col{b}")
            nc.vector.tensor_copy(ct, cps)
            c0_col.append(ct)

    # --- pass 2: out = W x + c0 1^T, fp16, stores on scalar queue --------
    with tc.tile_pool(name="o_ps", bufs=8, space="PSUM") as ops, \
         tc.tile_pool(name="o_sb", bufs=3) as osb:
        nsub = OUTW // NT
        for i in range(NPIX // OUTW):
            xj = (i * OUTW) // XCHUNK
            xo = (i * OUTW) % XCHUNK
            for b in range(2):
                ot = osb.tile([P, OUTW], F16, name="ot", tag="ot")
                for t in range(nsub):
                    pst = ops.tile([P, NT], F32, name="pst", tag="pst")
                    for k in range(2):
                        nc.tensor.matmul(
                            pst,
                            wt16[k][:, b * P:(b + 1) * P],
                            xs[k][xj][:, xo + t * NT:xo + (t + 1) * NT],
                            start=(k == 0),
                            stop=(k == 1),
                        )
                    # psum drain (bias add + fp16 cast) split across the
                    # otherwise-idle Scalar and Vector engines (GpSimd
                    # cannot read PSUM)
                    if t % 2 == 0:
                        nc.scalar.activation(
                            out=ot[:, t * NT:(t + 1) * NT], in_=pst,
                            func=mybir.ActivationFunctionType.Identity,
                            bias=c0_col[b], scale=1.0,
                        )
                    else:
                        nc.vector.tensor_scalar_add(
                            ot[:, t * NT:(t + 1) * NT], pst, c0_col[b],
                        )
                # store via SWDGE on the idle GpSimd engine so the Scalar
                # engine's cycle budget stays dedicated to psum drains;
                # the final stage stores in halves to shorten the tail
                if i == NPIX // OUTW - 1:
                    for hh in range(2):
                        nc.gpsimd.dma_start(
                            out=out_d[b * P:(b + 1) * P,
                                      i * OUTW + hh * (OUTW // 2):
                                      i * OUTW + (hh + 1) * (OUTW // 2)],
                            in_=ot[:, hh * (OUTW // 2):(hh + 1) * (OUTW // 2)],
                        )
                else:
                    nc.gpsimd.dma_start(
                        out=out_d[b * P:(b + 1) * P,
                                  i * OUTW:(i + 1) * OUTW],
                        in_=ot,
                    )


def build_program(enable_asserts=False):
    nc = bacc.Bacc(
        "TRN2",
        target_bir_lowering=False,
        debug=False,
        enable_asserts=enable_asserts,
        num_devices=8,
    )
    d_in = {
        "xht": nc.dram_tensor("xht", [NPIECE, P, CH_PP, C + 1],
                              F16, kind="ExternalInput").ap(),
        "xh": nc.dram_tensor("xh", [C, NPIX], F16,
                             kind="ExternalInput").ap(),
        "wcat": nc.dram_tensor("wcat", [C, 2 * C], F32,
                               kind="ExternalInput").ap(),
        "dmat": nc.dram_tensor("dmat", [C, C], F16,
                               kind="ExternalInput").ap(),
        "brows": nc.dram_tensor("brows", [3, C], F32,
                                kind="ExternalInput").ap(),
        "bcols": nc.dram_tensor("bcols", [C, 1], F16,
                                kind="ExternalInput").ap(),
        "ident": nc.dram_tensor("ident", [P, P], F32,
                                kind="ExternalInput").ap(),
    }
    d_out = {
        "out": nc.dram_tensor("out", [C, NPIX], F16,
                              kind="ExternalOutput").ap(),
    }
    with tile.TileContext(nc) as tc, ExitStack() as ctx:
        _emit(nc, tc, ctx, d_in, d_out)
    nc.compile()
    return nc


def make_in_maps(a, b, w1, b1, w2, b2, w3, b3, w4, b4):
    N = NPIX
    f = np.float32
    f64 = np.float64
    A = (w2.astype(f64) @ w1.astype(f64))
    B_ = (w3.astype(f64) @ w1.astype(f64))
    D = (w4.astype(f64) @ w1.astype(f64))
    p = (w2.astype(f64) @ b1.astype(f64) + b2)
    r = (w3.astype(f64) @ b1.astype(f64) + b3)
    t = (w4.astype(f64) @ b1.astype(f64) + b4)
    wcat = np.concatenate([A.T, B_.T], axis=1).astype(f)
    dmat = D.astype(np.float16)
    brows = np.stack([p, r, N * r]).astype(f)
    bcols = t[:, None].astype(np.float16)
    ident = np.eye(P, dtype=f)
    B = a.shape[0]
    in_maps = []
    for i in range(B):
        x = np.concatenate([a[i].reshape(P, N), b[i].reshape(P, N)], axis=0)
        xh = x.astype(np.float16)
        xht = np.ascontiguousarray(
            xh.T.reshape(NPIECE, CH_PP, P, C).transpose(0, 2, 1, 3))
        ones = np.ones((NPIECE, P, CH_PP, 1), np.float16)
        xht = np.ascontiguousarray(np.concatenate([xht, ones], axis=3))
        in_maps.append({
            "xht": xht,
            "xh": xh,
            "wcat": wcat,
            "dmat": dmat,
            "brows": brows,
            "bcols": bcols,
            "ident": ident,
        })
    return in_maps


_CACHE = {}


def kernel(a, b, w1, b1, w2, b2, w3, b3, w4, b4, _trace=False):
    a = np.asarray(a, dtype=np.float32)
    b = np.asarray(b, dtype=np.float32)
    args = [np.asarray(t, dtype=np.float32)
            for t in (w1, b1, w2, b2, w3, b3, w4, b4)]
    if "nc" not in _CACHE:
        _CACHE["nc"] = build_program()
    nc = _CACHE["nc"]
    in_maps = make_in_maps(a, b, *args)
    res = run_bass_kernel_spmd(nc, in_maps, core_ids=list(range(8)),
                               trace=_trace)
    B, Ch, H, W = a.shape
    out = np.stack([
        r["out"].astype(np.float32).reshape(C, H, W) for r in res.results
    ])
    if _trace:
        _CACHE["last_results"] = res
    return out


# revision 22
# speedup vs baseline: 1.3100x; 1.0259x over previous
"""Trainium2 Bass kernel for nn_CLF_block (channel-attention block).

Reference computation (per batch item i, with x = concat([a,b], ch) in [256, N],
N = H*W = 16384):
    z  = w1 x + b1 1^T
    q  = w2 z + b2 1^T ;  k = w3 z + b3 1^T ;  v = w4 z + b4 1^T
    qk = q k^T ; attn = softmax(qk, -1) ; out = attn v

Host-side weight folding (free: runs in numpy inside kernel()):
    q = A x + p 1^T   with A = w2 w1, p = w2 b1 + b2
    k = B x + r 1^T   with B = w3 w1, r = w3 b1 + b3
    v = D x + t 1^T   with D = w4 w1, t = w4 b1 + b4
so with Gx = x x^T (symmetric) and sx = x 1:
    qk   = A Gx B^T + (A sx) r^T + p (B sx + N r)^T
    attn = softmax(qk)
    out  = (attn D) x + (attn t) 1^T = W x + c0 1^T

Numerics: x is rounded to fp16 on host; Gx accumulates fp16 products in f32
(PSUM); the A.Gx.B^T sandwich runs in f32 (LOW_HIGH); the rank-1/vector
algebra, W and the pass-2 matmul run in fp16; the softmax normalization is
folded into the attn transposes as a diag(1/denom) stationary operand; the
output is stored as fp16 (upcast on host). Measured end-to-end error vs the
f64 reference: ~2.9e-3 max-rel (tolerance 2e-2).

Per-core HBM traffic: 8.4 MiB x^T stream (pass 1) + 8.4 MiB resident x
(pass 2) + 8.4 MiB output + ~1.2 MiB weights ~= 26.4 MiB -> memory-bound.
Schedule: the piece stream owns the sync DMA queue end to end (constants
ride the scalar queue in the startup DMA-idle window, the resident x
follows the stream on sync, output stores go via SWDGE on GpSimd). PE is
kept HAM-warm with startup matmuls on a zeroed tile; pass-2 psum drains are
split across the Scalar and Vector engines.

Sharding: data-parallel over batch, one batch item per NeuronCore (B=8).
"""

import sys

if "/opt/trn_rl_repo" not in sys.path:
    sys.path.insert(0, "/opt/trn_rl_repo")

from contextlib import ExitStack

import numpy as np

import concourse.bass as bass
import concourse.mybir as mybir
import concourse.tile as tile
from concourse import bacc
from concourse.bass_utils import run_bass_kernel_spmd

F32 = mybir.dt.float32
F16 = mybir.dt.float16
P = 128            # partitions / channel block
C = 256            # channels
NPIX = 128 * 128   # spatial positions per batch item
NPIECE = 16        # x^T stream pieces
CH_PP = 8          # gram chunks per piece
NCHUNK = NPIECE * CH_PP   # 128 gram chunks
XCHUNK = NPIX // 2        # resident x DMA chunk width
OUTW = 4096        # output staging tile width
NT = 512           # pass-2 psum tile width

# packed fp16 constant layout (columns)
W16_AT = 0          # A^T fp16, 2 row-blocks side by side   [0, 512)
W16_BT = 512        # B^T fp16                               [512, 1024)
W16_D = 1024        # D fp16                                 [1024, 1536)
W16_T = 1536        # t column fp16                          [1536, 1538)
W16_P = 1538        # p row (partition 0)                    [1538, 1794)
W16_R = 1794        # r row                                  [1794, 2050)
W16_NR = 2050       # N*r row                                [2050, 2306)
W16_W = 2306
# packed f32 constant layout
W32_AT = 0          # A^T f32                                [0, 512)
W32_BT = 512        # B^T f32                                [512, 1024)
W32_I = 1024        # identity                               [1024, 1152)
W32_P = 1152        # p row f32 (partition 0)                [1152, 1408)
W32_NR = 1408       # N*r row f32                            [1408, 1664)
W32_W = 1664


def _emit(nc, tc, ctx, d_in, d_out):
    """Emit the Tile program for one core (one batch item)."""
    xht_d, xh_d = d_in["xht"], d_in["xh"]
    wc32_d, wc16_d = d_in["wc32"], d_in["wc16"]
    out_d = d_out["out"]

    const = ctx.enter_context(tc.tile_pool(name="const", bufs=1))
    xpool = ctx.enter_context(tc.tile_pool(name="xpool", bufs=1))

    # --- PE warm-up: ~3.4us of matmuls on a zeroed tile so the HAM clock
    # gate is already released when the first stream piece lands.
    warm16 = const.tile([P, C], F16, name="warm16", tag="warm16")
    nc.vector.memset(warm16, 0.0)
    with tc.tile_pool(name="warm_ps0", bufs=1, space="PSUM") as wps0:
        wp = wps0.tile([P, C], F32, name="wp0", tag="wp0")
        for _ in range(16):
            nc.tensor.matmul(wp, warm16[:, 0:P], warm16,
                             start=True, stop=True)

    # --- constants: two packed DMAs on the scalar queue, issued first so
    # they transfer during the startup DMA-idle window and never contend
    # with the pass-1 stream
    wc32 = const.tile([P, W32_W], F32, name="wc32", tag="wc32")
    nc.scalar.dma_start(out=wc32, in_=wc32_d[:, :])
    wc16 = const.tile([P, W16_W], F16, name="wc16", tag="wc16")
    nc.scalar.dma_start(out=wc16, in_=wc16_d[:, :])

    at_ = [wc32[:, W32_AT + k * C:W32_AT + (k + 1) * C] for k in range(2)]
    bt_ = [wc32[:, W32_BT + k * C:W32_BT + (k + 1) * C] for k in range(2)]
    ident_sb = wc32[:, W32_I:W32_I + P]
    at16 = [wc16[:, W16_AT + k * C:W16_AT + (k + 1) * C] for k in range(2)]
    bt16 = [wc16[:, W16_BT + k * C:W16_BT + (k + 1) * C] for k in range(2)]
    dm_ = [wc16[:, W16_D + k * C:W16_D + (k + 1) * C] for k in range(2)]
    tcol = [wc16[:, W16_T + k:W16_T + k + 1] for k in range(2)]
    p_row = wc16[0:1, W16_P:W16_P + C]
    r_row = wc16[0:1, W16_R:W16_R + C]
    p32_row = wc32[0:1, W32_P:W32_P + C]
    nr32_row = wc32[0:1, W32_NR:W32_NR + C]

    # preload the EXP activation table so the softmax doesn't pay the
    # 1.3us ACT_TABLE_LOAD on the critical path
    warm_act = const.tile([P, 4], F32, name="warm_act", tag="warm_act")
    nc.scalar.activation(out=warm_act, in_=warm16[:, 0:4],
                         func=mybir.ActivationFunctionType.Exp, bias=0.0)

    # --- pass-1 stream: piece 0 split in half for an earlier first matmul;
    # the sync queue carries only the stream + the resident x
    xtp = ctx.enter_context(tc.tile_pool(name="xt_sb", bufs=5))
    H_PP = CH_PP // 2
    xh0 = []
    for h in range(2):
        xt = const.tile([P, H_PP, C + 1], F16, name=f"xh0_{h}",
                        tag=f"xh0_{h}")
        nc.sync.dma_start(out=xt, in_=xht_d[0][:, h * H_PP:(h + 1) * H_PP, :])
        xh0.append(xt)
    xht_p = [None]
    for i in range(1, 4):
        xt = xtp.tile([P, CH_PP, C + 1], F16, name="xht_p", tag="xht_p")
        nc.sync.dma_start(out=xt, in_=xht_d[i])
        xht_p.append(xt)

    # --- pass 1: Gx = xh xh^T (fp16 products, f32 accumulation) ----------
    # shh[b] accumulates rows b*128:(b+1)*128 of [Gx | sx] over all chunks.
    gx_sb = [
        const.tile([P, C + 1], F32, name=f"gx_sb{b}", tag=f"gx_sb{b}")
        for b in range(2)
    ]
    with tc.tile_pool(name="gx_ps", bufs=1, space="PSUM") as gxp:
        shh = [
            gxp.tile([P, C + 1], F32, name=f"shh{b}", tag=f"shh{b}")
            for b in range(2)
        ]
        for i in range(NPIECE):
            if i >= 4:
                xt = xtp.tile([P, CH_PP, C + 1], F16, name="xht_p",
                              tag="xht_p")
                nc.sync.dma_start(out=xt, in_=xht_d[i])
                xht_p.append(xt)
            for g in range(CH_PP):
                ch = i * CH_PP + g
                src_t = (xh0[g // H_PP][:, g % H_PP] if i == 0
                         else xht_p[i][:, g])
                for b in range(2):
                    nc.tensor.matmul(shh[b],
                                     src_t[:, b * P:(b + 1) * P],
                                     src_t,
                                     start=(ch == 0),
                                     stop=(ch == NCHUNK - 1))
        # resident x for pass 2, after the stream on the same queue
        xs = [[], []]
        for j in range(2):
            for k in range(2):
                xr = xpool.tile([P, XCHUNK], F16, name=f"x{k}_{j}",
                                tag=f"x{k}_{j}")
                nc.sync.dma_start(
                    out=xr,
                    in_=xh_d[k * P:(k + 1) * P,
                             j * XCHUNK:(j + 1) * XCHUNK])
                xs[k].append(xr)
        # small fp16 sx columns first so asx/bsx matmuls start immediately;
        # the big Gx copies run on Scalar and Vector in parallel
        sxc = []
        for b in range(2):
            sc = const.tile([P, 1], F16, name=f"sxc{b}", tag=f"sxc{b}")
            nc.vector.tensor_copy(sc, shh[b][:, C:C + 1])
            sxc.append(sc)
        nc.scalar.activation(out=gx_sb[0], in_=shh[0],
                             func=mybir.ActivationFunctionType.Identity,
                             bias=0.0, scale=1.0)
        nc.vector.tensor_copy(gx_sb[1], shh[1])

    # --- 256x256 algebra --------------------------------------------------
    alg = const
    with tc.tile_pool(name="alg_ps", bufs=3, space="PSUM") as ap:
        # asx_row = (A sx)^T, bsx_row = (B sx)^T  (fp16 matvecs)
        asx_row = alg.tile([1, C], F16, name="asx_row", tag="asx_row")
        bsx_row = alg.tile([1, C], F16, name="bsx_row", tag="bsx_row")
        for dst, wt in ((asx_row, at16), (bsx_row, bt16)):
            vps = ap.tile([1, C], F32, name="vps", tag="algsmall", bufs=2)
            for k in range(2):
                nc.tensor.matmul(vps, sxc[k], wt[k],
                                 start=(k == 0), stop=(k == 1))
            nc.vector.tensor_copy(dst, vps)

        # S = Gx B^T (Gx symmetric: lhsT = Gx row-blocks)
        s_sb = []
        for b in range(2):
            sps = ap.tile([P, C], F32, name="sps", tag="alg")
            for k in range(2):
                nc.tensor.matmul(sps, gx_sb[k][:, b * P:(b + 1) * P],
                                 bt_[k], start=(k == 0), stop=(k == 1))
            st = alg.tile([P, C], F32, name=f"s_sb{b}", tag=f"s_sb{b}")
            nc.vector.tensor_copy(st, sps)
            s_sb.append(st)

        # qk = A S + asx r^T + p (bsx + N r)^T ; softmax rows
        attn_sb = []
        for b in range(2):
            qkps = ap.tile([P, C], F32, name="qkps", tag="alg")
            for k in range(2):
                nc.tensor.matmul(qkps, at_[k][:, b * P:(b + 1) * P],
                                 s_sb[k], start=(k == 0), stop=False)
            nc.tensor.matmul(qkps, asx_row[:, b * P:(b + 1) * P], r_row,
                             start=False, stop=False)
            nc.tensor.matmul(qkps, p_row[:, b * P:(b + 1) * P], bsx_row,
                             start=False, stop=False)
            # the N p r^T term is ~+-57 in qk; fp16 rounding of it would
            # inject ~3e-2 noise, so it stays f32
            nc.tensor.matmul(qkps, p32_row[:, b * P:(b + 1) * P], nr32_row,
                             start=False, stop=True)

            negmax = alg.tile([P, 1], F32, name=f"negmax{b}", tag=f"nm{b}")
            nc.vector.tensor_reduce(
                out=negmax, in_=qkps, op=mybir.AluOpType.max,
                axis=mybir.AxisListType.X, negate=True,
            )
            expq = alg.tile([P, C], F32, name=f"expq{b}", tag=f"expq{b}")
            denom = alg.tile([P, 1], F32, name=f"denom{b}", tag=f"dn{b}")
            nc.scalar.activation(
                out=expq, in_=qkps, func=mybir.ActivationFunctionType.Exp,
                bias=negmax, scale=1.0, accum_out=denom,
            )
            rden = alg.tile([P, 1], F32, name=f"rden{b}", tag=f"rd{b}")
            nc.vector.reciprocal(rden, denom)
            # diag(1/denom): the transposing matmuls below fold the softmax
            # normalization into their stationary operand for free
            dident = alg.tile([P, P], F32, name=f"dident{b}", tag=f"di{b}")
            nc.vector.tensor_scalar_mul(dident, ident_sb, rden)
            attn_sb.append((expq, dident))

        # attn^T via 4 scaled-transpose matmuls, stored fp16
        attnT_sb = [
            alg.tile([P, C], F16, name=f"attnT{j}", tag=f"attnT{j}")
            for j in range(2)
        ]
        for b in range(2):
            expq_b, dident_b = attn_sb[b]
            for j in range(2):
                tps = ap.tile([P, P], F32, name="tps", tag="algtp", bufs=2)
                nc.tensor.matmul(tps, expq_b[:, j * P:(j + 1) * P],
                                 dident_b, start=True, stop=True)
                if j == 0:
                    nc.scalar.activation(
                        out=attnT_sb[j][:, b * P:(b + 1) * P], in_=tps,
                        func=mybir.ActivationFunctionType.Identity,
                        bias=0.0, scale=1.0)
                else:
                    nc.vector.tensor_copy(
                        attnT_sb[j][:, b * P:(b + 1) * P], tps)

        # W^T = D^T attn^T (fp16), cast immediately per block so pass 2
        # can start before the c0 matvecs retire
        wt16 = []
        for b in range(2):
            wps = ap.tile([P, C], F32, name="wps", tag="alg")
            for k in range(2):
                nc.tensor.matmul(wps, dm_[k][:, b * P:(b + 1) * P],
                                 attnT_sb[k], start=(k == 0), stop=(k == 1))
            wt_ = alg.tile([P, C], F16, name=f"wt16_{b}", tag=f"wt16_{b}")
            if b == 0:
                nc.scalar.activation(
                    out=wt_, in_=wps,
                    func=mybir.ActivationFunctionType.Identity,
                    bias=0.0, scale=1.0)
            else:
                nc.vector.tensor_copy(wt_, wps)
            wt16.append(wt_)

        # c0 = attn t (per q block)
        c0_col = []
        for b in range(2):
            cps = ap.tile([P, 1], F32, name="cps", tag="algsmall", bufs=2)
            for k in range(2):
                nc.tensor.matmul(cps, attnT_sb[k][:, b * P:(b + 1) * P],
                                 tcol[k], start=(k == 0), stop=(k == 1))
            ct = alg.tile([P, 1], F32, name=f"c0_col{b}", tag=f"c0_col{b}")
            nc.vector.tensor_copy(ct, cps)
            c0_col.append(ct)

    # --- pass 2: out = W x + c0 1^T, fp16, stores via SWDGE --------------
    with tc.tile_pool(name="o_ps", bufs=8, space="PSUM") as ops, \
         tc.tile_pool(name="o_sb", bufs=3) as osb:
        nsub = OUTW // NT
        for i in range(NPIX // OUTW):
            xj = (i * OUTW) // XCHUNK
            xo = (i * OUTW) % XCHUNK
            for b in range(2):
                ot = osb.tile([P, OUTW], F16, name="ot", tag="ot")
                for t in range(nsub):
                    pst = ops.tile([P, NT], F32, name="pst", tag="pst")
                    for k in range(2):
                        nc.tensor.matmul(
                            pst,
                            wt16[k][:, b * P:(b + 1) * P],
                            xs[k][xj][:, xo + t * NT:xo + (t + 1) * NT],
                            start=(k == 0),
                            stop=(k == 1),
                        )
                    # psum drain (bias add + fp16 cast) split across the
                    # otherwise-idle Scalar and Vector engines (GpSimd
                    # cannot read PSUM)
                    if t % 2 == 0:
                        nc.scalar.activation(
                            out=ot[:, t * NT:(t + 1) * NT], in_=pst,
                            func=mybir.ActivationFunctionType.Identity,
                            bias=c0_col[b], scale=1.0,
                        )
                    else:
                        nc.vector.tensor_scalar_add(
                            ot[:, t * NT:(t + 1) * NT], pst, c0_col[b],
                        )
                # store via SWDGE on the idle GpSimd engine so the Scalar
                # engine's cycle budget stays dedicated to psum drains;
                # the final stage stores in halves to shorten the tail
                if i == NPIX // OUTW - 1:
                    for hh in range(2):
                        nc.gpsimd.dma_start(
                            out=out_d[b * P:(b + 1) * P,
                                      i * OUTW + hh * (OUTW // 2):
                                      i * OUTW + (hh + 1) * (OUTW // 2)],
                            in_=ot[:, hh * (OUTW // 2):(hh + 1) * (OUTW // 2)],
                        )
                else:
                    nc.gpsimd.dma_start(
                        out=out_d[b * P:(b + 1) * P,
                                  i * OUTW:(i + 1) * OUTW],
                        in_=ot,
                    )


def build_program(enable_asserts=False):
    nc = bacc.Bacc(
        "TRN2",
        target_bir_lowering=False,
        debug=False,
        enable_asserts=enable_asserts,
        num_devices=8,
    )
    d_in = {
        "xht": nc.dram_tensor("xht", [NPIECE, P, CH_PP, C + 1],
                              F16, kind="ExternalInput").ap(),
        "xh": nc.dram_tensor("xh", [C, NPIX], F16,
                             kind="ExternalInput").ap(),
        "wc32": nc.dram_tensor("wc32", [P, W32_W], F32,
                               kind="ExternalInput").ap(),
        "wc16": nc.dram_tensor("wc16", [P, W16_W], F16,
                               kind="ExternalInput").ap(),
    }
    d_out = {
        "out": nc.dram_tensor("out", [C, NPIX], F16,
                              kind="ExternalOutput").ap(),
    }
    with tile.TileContext(nc) as tc, ExitStack() as ctx:
        _emit(nc, tc, ctx, d_in, d_out)
    nc.compile()
    return nc


def make_in_maps(a, b, w1, b1, w2, b2, w3, b3, w4, b4):
    N = NPIX
    f = np.float32
    f64 = np.float64
    A = (w2.astype(f64) @ w1.astype(f64))
    B_ = (w3.astype(f64) @ w1.astype(f64))
    D = (w4.astype(f64) @ w1.astype(f64))
    p = (w2.astype(f64) @ b1.astype(f64) + b2)
    r = (w3.astype(f64) @ b1.astype(f64) + b3)
    t = (w4.astype(f64) @ b1.astype(f64) + b4)

    def blocks2(m):  # [256, 256] -> [128, 512] (two row-blocks side by side)
        return np.concatenate([m[0:P, :], m[P:2 * P, :]], axis=1)

    wc32 = np.zeros((P, W32_W), f)
    wc32[:, W32_AT:W32_AT + 2 * C] = blocks2(A.T.astype(f))
    wc32[:, W32_BT:W32_BT + 2 * C] = blocks2(B_.T.astype(f))
    wc32[:, W32_I:W32_I + P] = np.eye(P, dtype=f)
    wc32[0, W32_P:W32_P + C] = p.astype(f)
    wc32[0, W32_NR:W32_NR + C] = (N * r).astype(f)

    f16 = np.float16
    wc16 = np.zeros((P, W16_W), f16)
    wc16[:, W16_AT:W16_AT + 2 * C] = blocks2(A.T.astype(f16))
    wc16[:, W16_BT:W16_BT + 2 * C] = blocks2(B_.T.astype(f16))
    wc16[:, W16_D:W16_D + 2 * C] = blocks2(D.astype(f16))
    wc16[:, W16_T:W16_T + 2] = t.astype(f16).reshape(2, P).T
    wc16[0, W16_P:W16_P + C] = p.astype(f16)
    wc16[0, W16_R:W16_R + C] = r.astype(f16)
    wc16[0, W16_NR:W16_NR + C] = (N * r).astype(f16)

    B = a.shape[0]
    in_maps = []
    for i in range(B):
        x = np.concatenate([a[i].reshape(P, N), b[i].reshape(P, N)], axis=0)
        xh = x.astype(np.float16)
        xht = np.ascontiguousarray(
            xh.T.reshape(NPIECE, CH_PP, P, C).transpose(0, 2, 1, 3))
        ones = np.ones((NPIECE, P, CH_PP, 1), np.float16)
        xht = np.ascontiguousarray(np.concatenate([xht, ones], axis=3))
        in_maps.append({
            "xht": xht,
            "xh": xh,
            "wc32": wc32,
            "wc16": wc16,
        })
    return in_maps


_CACHE = {}


def kernel(a, b, w1, b1, w2, b2, w3, b3, w4, b4, _trace=False):
    a = np.asarray(a, dtype=np.float32)
    b = np.asarray(b, dtype=np.float32)
    args = [np.asarray(t, dtype=np.float32)
            for t in (w1, b1, w2, b2, w3, b3, w4, b4)]
    if "nc" not in _CACHE:
        _CACHE["nc"] = build_program()
    nc = _CACHE["nc"]
    in_maps = make_in_maps(a, b, *args)
    res = run_bass_kernel_spmd(nc, in_maps, core_ids=list(range(8)),
                               trace=_trace)
    B, Ch, H, W = a.shape
    out = np.stack([
        r["out"].astype(np.float32).reshape(C, H, W) for r in res.results
    ])
    if _trace:
        _CACHE["last_results"] = res
    return out


# revision 23
# speedup vs baseline: 1.3769x; 1.0511x over previous
"""Trainium2 Bass kernel for nn_CLF_block (channel-attention block).

Reference computation (per batch item i, with x = concat([a,b], ch) in [256, N],
N = H*W = 16384):
    z  = w1 x + b1 1^T
    q  = w2 z + b2 1^T ;  k = w3 z + b3 1^T ;  v = w4 z + b4 1^T
    qk = q k^T ; attn = softmax(qk, -1) ; out = attn v

Host-side weight folding (free: runs in numpy inside kernel()):
    q = A x + p 1^T   with A = w2 w1, p = w2 b1 + b2
    k = B x + r 1^T   with B = w3 w1, r = w3 b1 + b3
    v = D x + t 1^T   with D = w4 w1, t = w4 b1 + b4
so with Gx = x x^T (symmetric) and sx = x 1:
    qk   = A Gx B^T + (A sx) r^T + p (B sx + N r)^T
    attn = softmax(qk)
    out  = (attn D) x + (attn t) 1^T = W x + c0 1^T

Numerics: x is rounded to fp16 on host; Gx accumulates fp16 products in f32
(PSUM); the A.Gx.B^T sandwich runs in f32 (LOW_HIGH); the rank-1/vector
algebra, W and the pass-2 matmul run in fp16; the softmax normalization is
folded into the attn transposes as a diag(1/denom) stationary operand; the
output is stored as fp16 (upcast on host). Measured end-to-end error vs the
f64 reference: ~2.9e-3 max-rel (tolerance 2e-2).

Per-core HBM traffic: 8.4 MiB x^T stream (pass 1) + 8.4 MiB resident x
(pass 2) + 8.4 MiB output + ~1.2 MiB weights ~= 26.4 MiB -> memory-bound.
Schedule: the piece stream owns the sync DMA queue end to end (constants
ride the scalar queue in the startup DMA-idle window, the resident x
follows the stream on sync, output stores go via SWDGE on GpSimd). PE is
kept HAM-warm with startup matmuls on a zeroed tile; pass-2 psum drains are
split across the Scalar and Vector engines.

Sharding: data-parallel over batch, one batch item per NeuronCore (B=8).
"""

import sys

if "/opt/trn_rl_repo" not in sys.path:
    sys.path.insert(0, "/opt/trn_rl_repo")

from contextlib import ExitStack

import numpy as np

import concourse.bass as bass
import concourse.mybir as mybir
import concourse.tile as tile
from concourse import bacc
from concourse.bass_utils import run_bass_kernel_spmd

F32 = mybir.dt.float32
F16 = mybir.dt.float16
P = 128            # partitions / channel block
C = 256            # channels
NPIX = 128 * 128   # spatial positions per batch item
NPIECE = 16        # x^T stream pieces
CH_PP = 8          # gram chunks per piece
NCHUNK = NPIECE * CH_PP   # 128 gram chunks
XCHUNK = NPIX // 2        # resident x DMA chunk width
OUTW = 4096        # output staging tile width
NT = 512           # pass-2 psum tile width

# packed fp16 constant layout (columns): D | t | p | r
W16_D = 0
W16_T = 512
W16_P = 514
W16_R = 770
W16_W = 1026
# packed f32 constants: tensor a = A^T; tensor b = B^T | ident | p | N*r
W32A_W = 512
W32B_BT = 0
W32B_I = 512
W32B_P = 640
W32B_NR = 896
W32B_W = 1152


def _emit(nc, tc, ctx, d_in, d_out):
    """Emit the Tile program for one core (one batch item)."""
    xht_d, xh_d = d_in["xht"], d_in["xh"]
    wc32a_d, wc32b_d, wc16_d = d_in["wc32a"], d_in["wc32b"], d_in["wc16"]
    out_d = d_out["out"]

    const = ctx.enter_context(tc.tile_pool(name="const", bufs=1))
    xpool = ctx.enter_context(tc.tile_pool(name="xpool", bufs=1))

    # --- PE warm-up: ~3.4us of matmuls on a zeroed tile so the HAM clock
    # gate is already released when the first stream piece lands.
    warm16 = const.tile([P, C], F16, name="warm16", tag="warm16")
    nc.vector.memset(warm16, 0.0)
    with tc.tile_pool(name="warm_ps0", bufs=1, space="PSUM") as wps0:
        wp = wps0.tile([P, C], F32, name="wp0", tag="wp0")
        for _ in range(30):
            nc.tensor.matmul(wp, warm16[:, 0:P], warm16,
                             start=True, stop=True)

    # --- constants: three small DMAs interleaved INTO the sync stream
    # below (the stream has ~0.2us/piece of DMA slack, so thin inserts
    # hide; one big const DMA anywhere would stall pass-1 by ~2-4us).
    # The fp16 copies of A^T/B^T are derived on-chip by the idle DVE.
    wc32a = const.tile([P, W32A_W], F32, name="wc32a", tag="wc32a")
    wc32b = const.tile([P, W32B_W], F32, name="wc32b", tag="wc32b")
    wc16 = const.tile([P, W16_W], F16, name="wc16", tag="wc16")
    at16_t = const.tile([P, 2 * C], F16, name="at16_t", tag="at16_t")
    bt16_t = const.tile([P, 2 * C], F16, name="bt16_t", tag="bt16_t")

    at_ = [wc32a[:, k * C:(k + 1) * C] for k in range(2)]
    bt_ = [wc32b[:, W32B_BT + k * C:W32B_BT + (k + 1) * C] for k in range(2)]
    ident_sb = wc32b[:, W32B_I:W32B_I + P]
    p32_row = wc32b[0:1, W32B_P:W32B_P + C]
    nr32_row = wc32b[0:1, W32B_NR:W32B_NR + C]
    at16 = [at16_t[:, k * C:(k + 1) * C] for k in range(2)]
    bt16 = [bt16_t[:, k * C:(k + 1) * C] for k in range(2)]
    dm_ = [wc16[:, W16_D + k * C:W16_D + (k + 1) * C] for k in range(2)]
    tcol = [wc16[:, W16_T + k:W16_T + k + 1] for k in range(2)]
    p_row = wc16[0:1, W16_P:W16_P + C]
    r_row = wc16[0:1, W16_R:W16_R + C]

    # preload the EXP activation table so the softmax doesn't pay the
    # 1.3us ACT_TABLE_LOAD on the critical path
    warm_act = const.tile([P, 4], F32, name="warm_act", tag="warm_act")
    nc.scalar.activation(out=warm_act, in_=warm16[:, 0:4],
                         func=mybir.ActivationFunctionType.Exp, bias=0.0)

    # --- pass-1 stream: piece 0 split in half for an earlier first matmul;
    # the sync queue carries only the stream + the resident x
    xtp = ctx.enter_context(tc.tile_pool(name="xt_sb", bufs=5))
    H_PP = CH_PP // 2
    xh0 = []
    for h in range(2):
        xt = const.tile([P, H_PP, C + 1], F16, name=f"xh0_{h}",
                        tag=f"xh0_{h}")
        nc.sync.dma_start(out=xt, in_=xht_d[0][:, h * H_PP:(h + 1) * H_PP, :])
        xh0.append(xt)
    xht_p = [None]
    for i in range(1, 4):
        xt = xtp.tile([P, CH_PP, C + 1], F16, name="xht_p", tag="xht_p")
        nc.sync.dma_start(out=xt, in_=xht_d[i])
        xht_p.append(xt)

    # --- pass 1: Gx = xh xh^T (fp16 products, f32 accumulation) ----------
    # shh[b] accumulates rows b*128:(b+1)*128 of [Gx | sx] over all chunks.
    gx_sb = [
        const.tile([P, C + 1], F32, name=f"gx_sb{b}", tag=f"gx_sb{b}")
        for b in range(2)
    ]
    with tc.tile_pool(name="gx_ps", bufs=1, space="PSUM") as gxp:
        shh = [
            gxp.tile([P, C + 1], F32, name=f"shh{b}", tag=f"shh{b}")
            for b in range(2)
        ]
        for i in range(NPIECE):
            if i >= 4:
                xt = xtp.tile([P, CH_PP, C + 1], F16, name="xht_p",
                              tag="xht_p")
                nc.sync.dma_start(out=xt, in_=xht_d[i])
                xht_p.append(xt)
            if i == 4:
                nc.sync.dma_start(out=wc32a, in_=wc32a_d[:, :])
                nc.vector.tensor_copy(at16_t, wc32a)
            elif i == 6:
                nc.sync.dma_start(out=wc32b, in_=wc32b_d[:, :])
                nc.vector.tensor_copy(bt16_t, wc32b[:, 0:2 * C])
            elif i == 8:
                nc.sync.dma_start(out=wc16, in_=wc16_d[:, :])
            for g in range(CH_PP):
                ch = i * CH_PP + g
                src_t = (xh0[g // H_PP][:, g % H_PP] if i == 0
                         else xht_p[i][:, g])
                for b in range(2):
                    nc.tensor.matmul(shh[b],
                                     src_t[:, b * P:(b + 1) * P],
                                     src_t,
                                     start=(ch == 0),
                                     stop=(ch == NCHUNK - 1))
        # resident x for pass 2, after the stream on the same queue
        xs = [[], []]
        for j in range(2):
            for k in range(2):
                xr = xpool.tile([P, XCHUNK], F16, name=f"x{k}_{j}",
                                tag=f"x{k}_{j}")
                nc.sync.dma_start(
                    out=xr,
                    in_=xh_d[k * P:(k + 1) * P,
                             j * XCHUNK:(j + 1) * XCHUNK])
                xs[k].append(xr)
        # small fp16 sx columns first so asx/bsx matmuls start immediately;
        # the big Gx copies run on Scalar and Vector in parallel
        sxc = []
        for b in range(2):
            sc = const.tile([P, 1], F16, name=f"sxc{b}", tag=f"sxc{b}")
            nc.vector.tensor_copy(sc, shh[b][:, C:C + 1])
            sxc.append(sc)
        nc.scalar.activation(out=gx_sb[0], in_=shh[0],
                             func=mybir.ActivationFunctionType.Identity,
                             bias=0.0, scale=1.0)
        nc.vector.tensor_copy(gx_sb[1], shh[1])

    # --- 256x256 algebra --------------------------------------------------
    alg = const
    with tc.tile_pool(name="alg_ps", bufs=3, space="PSUM") as ap:
        wp_alg = ap.tile([P, C], F32, name="wp_alg", tag="warm", bufs=1)
        # asx_row = (A sx)^T, bsx_row = (B sx)^T  (fp16 matvecs)
        asx_row = alg.tile([1, C], F16, name="asx_row", tag="asx_row")
        bsx_row = alg.tile([1, C], F16, name="bsx_row", tag="bsx_row")
        for dst, wt in ((asx_row, at16), (bsx_row, bt16)):
            vps = ap.tile([1, C], F32, name="vps", tag="algsmall", bufs=2)
            for k in range(2):
                nc.tensor.matmul(vps, sxc[k], wt[k],
                                 start=(k == 0), stop=(k == 1))
            nc.vector.tensor_copy(dst, vps)

        # S = Gx B^T (Gx symmetric: lhsT = Gx row-blocks)
        s_sb = []
        for b in range(2):
            sps = ap.tile([P, C], F32, name="sps", tag="alg")
            for k in range(2):
                nc.tensor.matmul(sps, gx_sb[k][:, b * P:(b + 1) * P],
                                 bt_[k], start=(k == 0), stop=(k == 1))
            st = alg.tile([P, C], F32, name=f"s_sb{b}", tag=f"s_sb{b}")
            nc.vector.tensor_copy(st, sps)
            s_sb.append(st)

        # qk = A S + asx r^T + p (bsx + N r)^T ; softmax rows
        attn_sb = []
        for b in range(2):
            qkps = ap.tile([P, C], F32, name="qkps", tag="alg")
            for k in range(2):
                nc.tensor.matmul(qkps, at_[k][:, b * P:(b + 1) * P],
                                 s_sb[k], start=(k == 0), stop=False)
            nc.tensor.matmul(qkps, asx_row[:, b * P:(b + 1) * P], r_row,
                             start=False, stop=False)
            nc.tensor.matmul(qkps, p_row[:, b * P:(b + 1) * P], bsx_row,
                             start=False, stop=False)
            # the N p r^T term is ~+-57 in qk; fp16 rounding of it would
            # inject ~3e-2 noise, so it stays f32
            nc.tensor.matmul(qkps, p32_row[:, b * P:(b + 1) * P], nr32_row,
                             start=False, stop=True)
            if b == 1:
                # cheap fp16 fills so the PE never idles a full HAM window
                # while the softmax chain runs
                for _ in range(6):
                    nc.tensor.matmul(wp_alg, warm16[:, 0:P], warm16,
                                     start=True, stop=True)

            negmax = alg.tile([P, 1], F32, name=f"negmax{b}", tag=f"nm{b}")
            nc.vector.tensor_reduce(
                out=negmax, in_=qkps, op=mybir.AluOpType.max,
                axis=mybir.AxisListType.X, negate=True,
            )
            expq = alg.tile([P, C], F32, name=f"expq{b}", tag=f"expq{b}")
            denom = alg.tile([P, 1], F32, name=f"denom{b}", tag=f"dn{b}")
            nc.scalar.activation(
                out=expq, in_=qkps, func=mybir.ActivationFunctionType.Exp,
                bias=negmax, scale=1.0, accum_out=denom,
            )
            rden = alg.tile([P, 1], F32, name=f"rden{b}", tag=f"rd{b}")
            nc.vector.reciprocal(rden, denom)
            # diag(1/denom): the transposing matmuls below fold the softmax
            # normalization into their stationary operand for free
            dident = alg.tile([P, P], F32, name=f"dident{b}", tag=f"di{b}")
            nc.vector.tensor_scalar_mul(dident, ident_sb, rden)
            attn_sb.append((expq, dident))

        # attn^T via 4 scaled-transpose matmuls, stored fp16
        attnT_sb = [
            alg.tile([P, C], F16, name=f"attnT{j}", tag=f"attnT{j}")
            for j in range(2)
        ]
        for b in range(2):
            expq_b, dident_b = attn_sb[b]
            for j in range(2):
                tps = ap.tile([P, P], F32, name="tps", tag="algtp", bufs=2)
                nc.tensor.matmul(tps, expq_b[:, j * P:(j + 1) * P],
                                 dident_b, start=True, stop=True)
                if j == 0:
                    nc.scalar.activation(
                        out=attnT_sb[j][:, b * P:(b + 1) * P], in_=tps,
                        func=mybir.ActivationFunctionType.Identity,
                        bias=0.0, scale=1.0)
                else:
                    nc.vector.tensor_copy(
                        attnT_sb[j][:, b * P:(b + 1) * P], tps)

        for _ in range(4):
            nc.tensor.matmul(wp_alg, warm16[:, 0:P], warm16,
                             start=True, stop=True)

        # W^T = D^T attn^T (fp16), cast immediately per block so pass 2
        # can start before the c0 matvecs retire
        wt16 = []
        for b in range(2):
            wps = ap.tile([P, C], F32, name="wps", tag="alg")
            for k in range(2):
                nc.tensor.matmul(wps, dm_[k][:, b * P:(b + 1) * P],
                                 attnT_sb[k], start=(k == 0), stop=(k == 1))
            wt_ = alg.tile([P, C], F16, name=f"wt16_{b}", tag=f"wt16_{b}")
            if b == 0:
                nc.scalar.activation(
                    out=wt_, in_=wps,
                    func=mybir.ActivationFunctionType.Identity,
                    bias=0.0, scale=1.0)
            else:
                nc.vector.tensor_copy(wt_, wps)
            wt16.append(wt_)

        # c0 = attn t (per q block)
        c0_col = []
        for b in range(2):
            cps = ap.tile([P, 1], F32, name="cps", tag="algsmall", bufs=2)
            for k in range(2):
                nc.tensor.matmul(cps, attnT_sb[k][:, b * P:(b + 1) * P],
                                 tcol[k], start=(k == 0), stop=(k == 1))
            ct = alg.tile([P, 1], F32, name=f"c0_col{b}", tag=f"c0_col{b}")
            nc.vector.tensor_copy(ct, cps)
            c0_col.append(ct)
        for _ in range(4):
            nc.tensor.matmul(wp_alg, warm16[:, 0:P], warm16,
                             start=True, stop=True)

    # --- pass 2: out = W x + c0 1^T, fp16, stores via SWDGE --------------
    with tc.tile_pool(name="o_ps", bufs=8, space="PSUM") as ops, \
         tc.tile_pool(name="o_sb", bufs=3) as osb:
        nsub = OUTW // NT
        for i in range(NPIX // OUTW):
            xj = (i * OUTW) // XCHUNK
            xo = (i * OUTW) % XCHUNK
            for b in range(2):
                ot = osb.tile([P, OUTW], F16, name="ot", tag="ot")
                for t in range(nsub):
                    pst = ops.tile([P, NT], F32, name="pst", tag="pst")
                    for k in range(2):
                        nc.tensor.matmul(
                            pst,
                            wt16[k][:, b * P:(b + 1) * P],
                            xs[k][xj][:, xo + t * NT:xo + (t + 1) * NT],
                            start=(k == 0),
                            stop=(k == 1),
                        )
                    # psum drain (bias add + fp16 cast) split across the
                    # otherwise-idle Scalar and Vector engines (GpSimd
                    # cannot read PSUM)
                    if t % 2 == 0:
                        nc.scalar.activation(
                            out=ot[:, t * NT:(t + 1) * NT], in_=pst,
                            func=mybir.ActivationFunctionType.Identity,
                            bias=c0_col[b], scale=1.0,
                        )
                    else:
                        nc.vector.tensor_scalar_add(
                            ot[:, t * NT:(t + 1) * NT], pst, c0_col[b],
                        )
                # store via SWDGE on the idle GpSimd engine so the Scalar
                # engine's cycle budget stays dedicated to psum drains;
                # the final stage stores in halves to shorten the tail
                if i == NPIX // OUTW - 1:
                    for hh in range(2):
                        nc.gpsimd.dma_start(
                            out=out_d[b * P:(b + 1) * P,
                                      i * OUTW + hh * (OUTW // 2):
                                      i * OUTW + (hh + 1) * (OUTW // 2)],
                            in_=ot[:, hh * (OUTW // 2):(hh + 1) * (OUTW // 2)],
                        )
                else:
                    nc.gpsimd.dma_start(
                        out=out_d[b * P:(b + 1) * P,
                                  i * OUTW:(i + 1) * OUTW],
                        in_=ot,
                    )


def build_program(enable_asserts=False):
    nc = bacc.Bacc(
        "TRN2",
        target_bir_lowering=False,
        debug=False,
        enable_asserts=enable_asserts,
        num_devices=8,
    )
    d_in = {
        "xht": nc.dram_tensor("xht", [NPIECE, P, CH_PP, C + 1],
                              F16, kind="ExternalInput").ap(),
        "xh": nc.dram_tensor("xh", [C, NPIX], F16,
                             kind="ExternalInput").ap(),
        "wc32a": nc.dram_tensor("wc32a", [P, W32A_W], F32,
                                kind="ExternalInput").ap(),
        "wc32b": nc.dram_tensor("wc32b", [P, W32B_W], F32,
                                kind="ExternalInput").ap(),
        "wc16": nc.dram_tensor("wc16", [P, W16_W], F16,
                               kind="ExternalInput").ap(),
    }
    d_out = {
        "out": nc.dram_tensor("out", [C, NPIX], F16,
                              kind="ExternalOutput").ap(),
    }
    with tile.TileContext(nc) as tc, ExitStack() as ctx:
        _emit(nc, tc, ctx, d_in, d_out)
    nc.compile()
    return nc


def make_in_maps(a, b, w1, b1, w2, b2, w3, b3, w4, b4):
    N = NPIX
    f = np.float32
    f64 = np.float64
    A = (w2.astype(f64) @ w1.astype(f64))
    B_ = (w3.astype(f64) @ w1.astype(f64))
    D = (w4.astype(f64) @ w1.astype(f64))
    p = (w2.astype(f64) @ b1.astype(f64) + b2)
    r = (w3.astype(f64) @ b1.astype(f64) + b3)
    t = (w4.astype(f64) @ b1.astype(f64) + b4)

    def blocks2(m):  # [256, 256] -> [128, 512] (two row-blocks side by side)
        return np.concatenate([m[0:P, :], m[P:2 * P, :]], axis=1)

    wc32a = np.ascontiguousarray(blocks2(A.T.astype(f)))
    wc32b = np.zeros((P, W32B_W), f)
    wc32b[:, W32B_BT:W32B_BT + 2 * C] = blocks2(B_.T.astype(f))
    wc32b[:, W32B_I:W32B_I + P] = np.eye(P, dtype=f)
    wc32b[0, W32B_P:W32B_P + C] = p.astype(f)
    wc32b[0, W32B_NR:W32B_NR + C] = (N * r).astype(f)

    f16 = np.float16
    wc16 = np.zeros((P, W16_W), f16)
    wc16[:, W16_D:W16_D + 2 * C] = blocks2(D.astype(f16))
    wc16[:, W16_T:W16_T + 2] = t.astype(f16).reshape(2, P).T
    wc16[0, W16_P:W16_P + C] = p.astype(f16)
    wc16[0, W16_R:W16_R + C] = r.astype(f16)

    B = a.shape[0]
    in_maps = []
    for i in range(B):
        x = np.concatenate([a[i].reshape(P, N), b[i].reshape(P, N)], axis=0)
        xh = x.astype(np.float16)
        xht = np.ascontiguousarray(
            xh.T.reshape(NPIECE, CH_PP, P, C).transpose(0, 2, 1, 3))
        ones = np.ones((NPIECE, P, CH_PP, 1), np.float16)
        xht = np.ascontiguousarray(np.concatenate([xht, ones], axis=3))
        in_maps.append({
            "xht": xht,
            "xh": xh,
            "wc32a": wc32a,
            "wc32b": wc32b,
            "wc16": wc16,
        })
    return in_maps


_CACHE = {}


def kernel(a, b, w1, b1, w2, b2, w3, b3, w4, b4, _trace=False):
    a = np.asarray(a, dtype=np.float32)
    b = np.asarray(b, dtype=np.float32)
    args = [np.asarray(t, dtype=np.float32)
            for t in (w1, b1, w2, b2, w3, b3, w4, b4)]
    if "nc" not in _CACHE:
        _CACHE["nc"] = build_program()
    nc = _CACHE["nc"]
    in_maps = make_in_maps(a, b, *args)
    res = run_bass_kernel_spmd(nc, in_maps, core_ids=list(range(8)),
                               trace=_trace)
    B, Ch, H, W = a.shape
    out = np.stack([
        r["out"].astype(np.float32).reshape(C, H, W) for r in res.results
    ])
    if _trace:
        _CACHE["last_results"] = res
    return out
